# revision 70
# baseline (speedup 1.0000x reference)
"""Trainium2 Bass kernel for nn_GATNodeScorer (GNN message passing).

Strategy (8 NeuronCores, node-partitioned):
  - Host: permute nodes into 160 balanced (core, tile) bins of 128 slots so
    every tile has <= K*128 in-edges; pack edges into 128-edge chunks per
    destination tile; fold attention projections and biases into augmented
    weight matrices.
  - Device, per core (SPMD, one NEFF):
      1. input projection  h = relu(xc @ Wp + bp)    (slab of 2560 nodes)
      2. AllGather H table (f32) across 8 cores
      3. relational layer  h1 = h + segsum(h[src] + rel_emb[type]*w)
         via per-chunk [P,1] indirect-DMA gathers + one-hot f32r matmul
         scatter-add
      4. dense x1 = h1 @ [W1 | W1@Asrc | W1@Adst] in f32r  -> XA table
         (bf16 x, attention logits as bf16 hi/lo pairs), AllGather
      5. GAT layer: per-chunk gathers of [x | as_hi | as_lo] rows by edge
         src; per-edge a_dst via transposed one-hot matmuls (exact f32);
         segment softmax via exp with the denominator columns merged into
         the numerator matmul rhs [msg(256) | ex(4)]; exp() is expanded
         across the 64 head channels on the ACT engine so the bf16 message
         scaling runs in DVE 2x mode
      6. repeat 4-5 for layer 2, then score = h3 @ Wo + bo
  - bf16 message data plane, f32r dense path, f32 PSUM accumulation.

Self-contained: hardcodes all shapes; only needs numpy + the concourse repo
installed at /opt/trn_rl_repo.
"""

import sys

sys.path.insert(0, "/opt/trn_rl_repo")

import heapq

import numpy as np
import ml_dtypes

import concourse.bass as bass
import concourse.bacc as bacc
import concourse.mybir as mybir
import concourse.tile as tile
from concourse.bass_utils import run_bass_kernel_spmd
from concourse.masks import make_identity

# ---- problem constants (hardcoded per contest rules) ----
N, E = 20000, 320000
IN_DIM, CODE_DIM, HIDDEN, HEADS, NREL = 896, 768, 256, 4, 5
CH = HIDDEN // HEADS
CODE_WEIGHT = 3.0
NEG_SLOPE = 0.2

NCORES = 8
P = 128
T = 20  # tiles per core
NTILES = NCORES * T  # 160
NP = T * P  # 2560 padded nodes per core
NPAD = NTILES * P  # 20480
KPROJ = IN_DIM // P  # 7

F32 = mybir.dt.float32
F32R = mybir.dt.float32r
BF16 = mybir.dt.bfloat16
I32 = mybir.dt.int32
NPBF = np.dtype(ml_dtypes.bfloat16)

XAW = HIDDEN + 2 * HEADS  # 264 dense output: [x | a_src | a_dst]
TBLW = HIDDEN + 2 * HEADS  # 264 shared table row: [x | as_hi | as_lo]
MW = HIDDEN + HEADS  # 260 merged matmul rhs: [msg | ex]
NRELP = 6  # NREL padded even

# ---------------------------------------------------------------------------
# host-side planning
# ---------------------------------------------------------------------------


def _pack_nodes(deg_gat, deg_rel, cap_g, cap_r):
    order = np.argsort(-deg_gat, kind="stable")
    load_g = np.zeros(NTILES, np.int64)
    load_r = np.zeros(NTILES, np.int64)
    count = np.zeros(NTILES, np.int64)
    tile_of = np.full(N, -1, np.int64)
    heap = [(0, t) for t in range(NTILES)]
    heapq.heapify(heap)
    for n in order:
        dg, dr = deg_gat[n], deg_rel[n]
        popped = []
        placed = False
        while heap:
            lg, t = heapq.heappop(heap)
            if lg != load_g[t]:
                continue
            if count[t] < P and load_g[t] + dg <= cap_g and load_r[t] + dr <= cap_r:
                tile_of[n] = t
                load_g[t] += dg
                load_r[t] += dr
                count[t] += 1
                if count[t] < P:
                    heapq.heappush(heap, (load_g[t], t))
                placed = True
                break
            popped.append((lg, t))
        for item in popped:
            heapq.heappush(heap, item)
        if not placed:
            raise RuntimeError("packing failed")
    return tile_of


def _pack_edges(src_pp, dst_pp, K):
    tile_e = dst_pp // P
    order_e = np.argsort(tile_e, kind="stable")
    esrc = np.zeros((NTILES, K * P), np.int32)
    dloc = np.full((NTILES, K * P), P, np.float32)
    eord = np.full((NTILES, K * P), -1, np.int64)
    bounds = np.searchsorted(tile_e[order_e], np.arange(NTILES + 1))
    for t in range(NTILES):
        lo, hi = bounds[t], bounds[t + 1]
        ecnt = hi - lo
        if ecnt > K * P:
            raise RuntimeError(f"tile {t}: {ecnt} edges > {K * P}")
        idxs = order_e[lo:hi]
        esrc[t, :ecnt] = src_pp[idxs]
        dloc[t, :ecnt] = (dst_pp[idxs] - t * P).astype(np.float32)
        eord[t, :ecnt] = idxs
    esrc = np.ascontiguousarray(esrc.reshape(NTILES, K, P).transpose(0, 2, 1))
    dloc = np.ascontiguousarray(dloc.reshape(NTILES, K, P).transpose(0, 2, 1))
    eord = np.ascontiguousarray(eord.reshape(NTILES, K, P).transpose(0, 2, 1))
    return esrc, dloc, eord


def _build_plan(edge_index):
    src = edge_index[0].astype(np.int64)
    dst = edge_index[1].astype(np.int64)
    deg_rel = np.bincount(dst, minlength=N)
    deg_gat = deg_rel + 1
    for K_G, K_R in ((17, 16), (18, 17), (19, 18)):
        try:
            tile_of = _pack_nodes(deg_gat, deg_rel, K_G * P, K_R * P)
            break
        except RuntimeError:
            continue
    else:
        raise RuntimeError("node packing failed at all K")

    perm = np.full(N, -1, np.int64)
    slot_ctr = np.zeros(NTILES, np.int64)
    for n in np.argsort(tile_of, kind="stable"):
        t = tile_of[n]
        perm[n] = t * P + slot_ctr[t]
        slot_ctr[t] += 1

    src_p, dst_p = perm[src], perm[dst]
    esrc_r, dloc_r, eord_r = _pack_edges(src_p, dst_p, K_R)
    loop = perm[np.arange(N)]
    esrc_g, dloc_g, _ = _pack_edges(
        np.concatenate([src_p, loop]), np.concatenate([dst_p, loop]), K_G
    )
    return dict(
        perm=perm,
        K_G=K_G,
        K_R=K_R,
        esrc_r=esrc_r,
        dloc_r=dloc_r,
        eord_r=eord_r,
        esrc_g=esrc_g,
        dloc_g=dloc_g,
    )


def _asrc_mat(att):
    """[HEADS, CH] -> [HIDDEN, HEADS] block matrix so x @ A == (x*att).sum(-1)."""
    A = np.zeros((HIDDEN, HEADS), np.float32)
    for h in range(HEADS):
        A[h * CH : (h + 1) * CH, h] = att[h]
    return A


# ---------------------------------------------------------------------------
# bass program
# ---------------------------------------------------------------------------


def _build_bass(K_R, K_G, probe=None):
    probe = probe or {}
    reps = probe.get("reps", 1)
    nc = bacc.Bacc("TRN2", target_bir_lowering=False, debug=False, num_devices=NCORES)

    # ---- external inputs ----
    xtt_in = nc.dram_tensor("xtt", [T, KPROJ, P, P], F32, kind="ExternalInput")
    wp_in = nc.dram_tensor("wp", [KPROJ, P, HIDDEN], F32, kind="ExternalInput")
    bp_in = nc.dram_tensor("bp_row", [1, HIDDEN], F32, kind="ExternalInput")
    w1_in = nc.dram_tensor("w1aug", [2, P, XAW], F32, kind="ExternalInput")
    w2_in = nc.dram_tensor("w2aug", [2, P, XAW], F32, kind="ExternalInput")
    b1w2_in = nc.dram_tensor("b1w2_row", [1, XAW], F32, kind="ExternalInput")
    rel_in = nc.dram_tensor("rel_emb", [NRELP, HIDDEN], F32, kind="ExternalInput")
    worep_in = nc.dram_tensor("wo_rep", [P, HIDDEN], F32, kind="ExternalInput")
    scb_in = nc.dram_tensor("sc_bias", [P, 1], F32, kind="ExternalInput")
    esrc_r_in = nc.dram_tensor("esrc_r", [T, P, K_R], I32, kind="ExternalInput")
    dloc_r_in = nc.dram_tensor("dloc_r", [T, P, K_R], BF16, kind="ExternalInput")
    wtyp_r_in = nc.dram_tensor("wtyp_r", [T, P, NRELP * K_R], BF16, kind="ExternalInput")
    esrc_g_in = nc.dram_tensor("esrc_g", [T, P, K_G], I32, kind="ExternalInput")
    dloc_g_in = nc.dram_tensor("dloc_g", [T, P, K_G], F32, kind="ExternalInput")

    score_out = nc.dram_tensor("score", [NP], F32, kind="ExternalOutput")

    with tile.TileContext(nc) as tc:
        with (
            tc.tile_pool(name="const", bufs=1) as cpool,
            tc.tile_pool(name="hres", bufs=1) as hpool,
            tc.tile_pool(name="lhsT", bufs=4) as lpool,
            tc.tile_pool(name="edge_idx", bufs=4) as epool,
            tc.tile_pool(name="gather", bufs=4) as gpool,
            tc.tile_pool(name="onehot", bufs=3) as opool,
            tc.tile_pool(name="msg", bufs=2) as mpool,
            tc.tile_pool(name="small", bufs=4) as spool,
            tc.tile_pool(name="ps", bufs=1, space="PSUM") as pspool,
            tc.tile_pool(name="dram", bufs=1, space="DRAM") as dpool,
        ):
            # ---- constants ----
            ident = cpool.tile([P, P], F32)
            make_identity(nc, ident[:])
            ident_bf = cpool.tile([P, P], BF16)
            nc.vector.tensor_copy(ident_bf[:], ident[:])
            iota_row_i = cpool.tile([P, P], I32)
            nc.gpsimd.iota(iota_row_i[:], pattern=[[1, P]], base=0, channel_multiplier=0)
            iota_row = cpool.tile([P, P], BF16)
            nc.vector.tensor_copy(iota_row[:], iota_row_i[:])
            iota_row_f = cpool.tile([P, P], F32)
            nc.vector.tensor_copy(iota_row_f[:], iota_row_i[:])
            iota_col_i = cpool.tile([P, 1], I32)
            nc.gpsimd.iota(iota_col_i[:], pattern=[[0, 1]], base=0, channel_multiplier=1)
            iota_col = cpool.tile([P, 1], F32)
            nc.vector.tensor_copy(iota_col[:], iota_col_i[:])
            ones_row = cpool.tile([1, P], F32)
            nc.vector.memset(ones_row[:], 1.0)

            # weights resident in SBUF (f32r: full-rate matmuls, ~1e-5 error)
            w_scr = cpool.tile([P, 2 * XAW], F32)
            wp_sb = cpool.tile([P, KPROJ * HIDDEN], F32R)
            for k in range(KPROJ):
                nc.sync.dma_start(w_scr[:, 0:HIDDEN], wp_in[k, :, :])
                nc.vector.tensor_copy(
                    wp_sb[:, k * HIDDEN : (k + 1) * HIDDEN], w_scr[:, 0:HIDDEN]
                )
            bp_sb = cpool.tile([1, HIDDEN], F32)
            nc.sync.dma_start(bp_sb[:], bp_in[:, :])

            waug = []
            for li, w_in in enumerate((w1_in, w2_in)):
                wr = cpool.tile([P, 2 * XAW], F32R, name=f"w{li}")
                for k in range(2):
                    nc.sync.dma_start(w_scr[:, k * XAW : (k + 1) * XAW], w_in[k, :, :])
                nc.vector.tensor_copy(wr[:], w_scr[:])
                waug.append(wr)

            b1w2_sb = cpool.tile([1, XAW], F32)
            nc.sync.dma_start(b1w2_sb[:], b1w2_in[:, :])
            rel_f = cpool.tile([NRELP, HIDDEN], F32)
            nc.sync.dma_start(rel_f[:], rel_in[:, :])
            rel_sb = cpool.tile([NRELP, HIDDEN], BF16)
            nc.vector.tensor_copy(rel_sb[:], rel_f[:])
            worep_sb = cpool.tile([P, HIDDEN], F32)
            nc.sync.dma_start(worep_sb[:], worep_in[:, :])
            scb_sb = cpool.tile([P, 1], F32)
            nc.sync.dma_start(scb_sb[:], scb_in[:, :])

            # residual h slabs (two ping-pong slabs of T tiles, f32)
            hA = hpool.tile([P, T * HIDDEN], F32)
            hB = hpool.tile([P, T * HIDDEN], F32)
            # resident per-tile a_dst columns (f32, exact)
            adst_all = hpool.tile([P, T * HEADS], F32)

            # DRAM bounce buffers for collectives (bf16).  A Shared tensor
            # may only be written by one instruction, so timing builds
            # (reps > 1) get per-rep tables.
            h_slab = dpool.tile([NP, HIDDEN], BF16)
            xa_slab = dpool.tile([NP, TBLW], BF16)
            xa_slab2 = dpool.tile([NP, TBLW], BF16)
            h_fulls = [
                dpool.tile([NPAD, HIDDEN], BF16, addr_space="Shared", name=f"h_full{r}")
                for r in range(reps)
            ]
            xa_fulls = [
                dpool.tile([NPAD, TBLW], BF16, addr_space="Shared", name=f"xa_full{r}")
                for r in range(reps)
            ]
            xa_full2s = [
                dpool.tile([NPAD, TBLW], BF16, addr_space="Shared", name=f"xa_full2{r}")
                for r in range(reps)
            ]

            def hcols(t):
                return slice(t * HIDDEN, (t + 1) * HIDDEN)

            for rep in range(reps):
                h_full = h_fulls[rep]
                xa_full = xa_fulls[rep]
                xa_full2 = xa_full2s[rep]
                # ================= stage 1: input projection =================
                for t in range(T):
                    proj_ps = pspool.tile([P, HIDDEN], F32, tag="work", bufs=2)
                    for k in range(KPROJ):
                        lx = lpool.tile([P, P], F32, tag="lhsT")
                        nc.sync.dma_start(lx[:], xtt_in[t, k, :, :])
                        lxr = lpool.tile([P, P], F32R, tag="lhsTr")
                        nc.vector.tensor_copy(lxr[:], lx[:])
                        nc.tensor.matmul(
                            out=proj_ps[:],
                            lhsT=lxr[:],
                            rhs=wp_sb[:, k * HIDDEN : (k + 1) * HIDDEN],
                            start=(k == 0),
                            stop=False,
                        )
                    nc.tensor.matmul(
                        out=proj_ps[:],
                        lhsT=ones_row[:1, :],
                        rhs=bp_sb[:1, :],
                        start=False,
                        stop=True,
                    )
                    nc.scalar.activation(
                        out=hA[:, hcols(t)],
                        in_=proj_ps[:],
                        func=mybir.ActivationFunctionType.Relu,
                    )
                    hsl = spool.tile([P, HIDDEN], BF16, tag="hsl")
                    nc.vector.tensor_copy(hsl[:], hA[:, hcols(t)])
                    nc.sync.dma_start(h_slab[t * P : (t + 1) * P, :], hsl[:])

                if probe.get("stop_after") == "proj":
                    continue
                # ================= AllGather H =================
                if probe.get("no_collective"):
                    nc.sync.dma_start(h_full[0:NP, :], h_slab[:, :])
                else:
                    nc.gpsimd.collective_compute(
                        "AllGather",
                        mybir.AluOpType.bypass,
                        replica_groups=[list(range(NCORES))],
                        ins=[h_slab.opt()],
                        outs=[h_full.opt()],
                    )

                # ================= stage 2: relational layer =================
                for t in range(T):
                    esrc_t = epool.tile([P, K_R], I32, tag="esrc")
                    nc.sync.dma_start(esrc_t[:], esrc_r_in[t, :, :])
                    dloc_t = epool.tile([P, K_R], BF16, tag="dlocb")
                    nc.sync.dma_start(dloc_t[:], dloc_r_in[t, :, :])
                    wt_t = epool.tile([P, NRELP * K_R], BF16, tag="wtyp")
                    nc.sync.dma_start(wt_t[:], wtyp_r_in[t, :, :])

                    # batched gather of all K_R chunks for this tile (bf16)
                    hch = gpool.tile([P, K_R * HIDDEN], BF16, tag="gather")
                    for k in range(K_R):
                        nc.gpsimd.indirect_dma_start(
                            out=hch[:, k * HIDDEN : (k + 1) * HIDDEN],
                            out_offset=None,
                            in_=h_full[:, :],
                            in_offset=bass.IndirectOffsetOnAxis(
                                ap=esrc_t[:, k : k + 1], axis=0
                            ),
                        )
                    # all one-hots in one DVE op
                    oh = opool.tile([P, K_R * P], BF16, tag="onehot")
                    nc.vector.tensor_tensor(
                        out=oh[:].rearrange("p (k e) -> p k e", k=K_R),
                        in0=dloc_t[:].unsqueeze(-1).to_broadcast([P, K_R, P]),
                        in1=iota_row[:].unsqueeze(1).to_broadcast([P, K_R, P]),
                        op=mybir.AluOpType.is_equal,
                    )
                    out_ps = pspool.tile([P, HIDDEN], F32, tag="out", bufs=2)
                    wm_ps = pspool.tile([P, NRELP], F32, tag="acc4", bufs=1)
                    for k in range(K_R):
                        nc.tensor.matmul(
                            out=out_ps[:],
                            lhsT=oh[:, k * P : (k + 1) * P],
                            rhs=hch[:, k * HIDDEN : (k + 1) * HIDDEN],
                            start=(k == 0),
                            stop=(k == K_R - 1),
                        )
                        nc.tensor.matmul(
                            out=wm_ps[:],
                            lhsT=oh[:, k * P : (k + 1) * P],
                            rhs=wt_t[:, k * NRELP : (k + 1) * NRELP],
                            start=(k == 0),
                            stop=(k == K_R - 1),
                        )
                    # rel contribution: wmatT [6, P] then rel_embT matmul
                    wmat_sb = spool.tile([P, NRELP], BF16, tag="wmat")
                    nc.vector.tensor_copy(wmat_sb[:], wm_ps[:])
                    wmatT_ps = pspool.tile([NRELP, P], BF16, tag="tmp", bufs=2)
                    nc.tensor.transpose(
                        out=wmatT_ps[:], in_=wmat_sb[:], identity=ident_bf[:]
                    )
                    wmatT_sb = spool.tile([NRELP, P], BF16, tag="wmatT")
                    nc.vector.tensor_copy(wmatT_sb[:], wmatT_ps[:])
                    rel_ps = pspool.tile([P, HIDDEN], F32, tag="work", bufs=2)
                    nc.tensor.matmul(
                        out=rel_ps[:],
                        lhsT=wmatT_sb[:],
                        rhs=rel_sb[:],
                        start=True,
                        stop=True,
                    )
                    # h1 = h + segsum + rel  (one PSUM operand per DVE op)
                    tsum = spool.tile([P, HIDDEN], F32, tag="tsum")
                    nc.vector.tensor_add(tsum[:], out_ps[:], hA[:, hcols(t)])
                    nc.vector.tensor_add(hB[:, hcols(t)], rel_ps[:], tsum[:])

                if probe.get("stop_after") == "rel":
                    continue
                # ============ stages 3/4: GAT layers ============
                for layer in range(2):
                    hin = hB if layer == 0 else hA
                    hout = hA if layer == 0 else hB
                    wr = waug[layer]
                    slab = xa_slab if layer == 0 else xa_slab2
                    full = xa_full if layer == 0 else xa_full2

                    # ---- dense: x = h @ Waug (+ b-fold for layer 1) ----
                    for t in range(T):
                        x_ps = pspool.tile([P, XAW], F32, tag="work", bufs=2)
                        for half in range(2):
                            tr_ps = pspool.tile([P, P], F32, tag="tmp", bufs=2)
                            nc.tensor.transpose(
                                out=tr_ps[:],
                                in_=hin[
                                    :,
                                    t * HIDDEN + half * P : t * HIDDEN + (half + 1) * P,
                                ],
                                identity=ident[:],
                            )
                            ht_r = lpool.tile([P, P], F32R, tag="lhsTr")
                            nc.vector.tensor_copy(ht_r[:], tr_ps[:])
                            nc.tensor.matmul(
                                out=x_ps[:],
                                lhsT=ht_r[:],
                                rhs=wr[:, half * XAW : (half + 1) * XAW],
                                start=(half == 0),
                                stop=(half == 1 and layer == 0),
                            )
                        if layer == 1:
                            # fold h2 = gat1_out + b1 into x2 = h2 @ W2aug
                            nc.tensor.matmul(
                                out=x_ps[:],
                                lhsT=ones_row[:1, :],
                                rhs=b1w2_sb[:1, :],
                                start=False,
                                stop=True,
                            )
                        # shared-table row [x(256) | as_hi | as_lo]; a_src is
                        # stored as a bf16 hi/lo split of the f32 logits
                        # (exp() amplifies rounding).  a_dst stays resident
                        # in f32 (only needed for local dst nodes).
                        xa_sb = gpool.tile([P, TBLW], BF16, tag="xa_sb")
                        nc.vector.tensor_copy(xa_sb[:, 0:HIDDEN], x_ps[:, 0:HIDDEN])
                        as_ps = x_ps[:, HIDDEN : HIDDEN + HEADS]
                        hi_ap = xa_sb[:, HIDDEN : HIDDEN + HEADS]
                        lo_ap = xa_sb[:, HIDDEN + HEADS : HIDDEN + 2 * HEADS]
                        nc.vector.tensor_copy(hi_ap, as_ps)
                        hi32 = spool.tile([P, HEADS], F32, tag="hi32")
                        nc.vector.tensor_copy(hi32[:], hi_ap)
                        nc.vector.tensor_tensor(
                            out=lo_ap, in0=as_ps, in1=hi32[:],
                            op=mybir.AluOpType.subtract,
                        )
                        nc.vector.tensor_copy(
                            adst_all[:, t * HEADS : (t + 1) * HEADS],
                            x_ps[:, HIDDEN + HEADS : XAW],
                        )
                        nc.sync.dma_start(slab[t * P : (t + 1) * P, :], xa_sb[:])

                    if probe.get("no_collective"):
                        nc.sync.dma_start(full[0:NP, :], slab[:, :])
                    else:
                        nc.gpsimd.collective_compute(
                            "AllGather",
                            mybir.AluOpType.bypass,
                            replica_groups=[list(range(NCORES))],
                            ins=[slab.opt()],
                            outs=[full.opt()],
                        )

                    # ---- edge stage ----
                    if probe.get("stop_after") == f"dense{layer + 1}":
                        break
                    for t in range(T):
                        esrc_t = epool.tile([P, K_G], I32, tag="esrc")
                        nc.sync.dma_start(esrc_t[:], esrc_g_in[t, :, :])
                        dloc_t = epool.tile([P, K_G], F32, tag="dloc")
                        nc.sync.dma_start(dloc_t[:], dloc_g_in[t, :, :])
                        dloc_bf = epool.tile([P, K_G], BF16, tag="dlocb")
                        nc.vector.tensor_copy(dloc_bf[:], dloc_t[:])

                        # batched gather: [x | as_hi | as_lo] rows by src
                        xa_all = gpool.tile([P, K_G * TBLW], BF16, tag="gather")
                        xa_v = xa_all[:].rearrange("p (k w) -> p k w", k=K_G)
                        for k in range(K_G):
                            nc.gpsimd.indirect_dma_start(
                                out=xa_all[:, k * TBLW : (k + 1) * TBLW],
                                out_offset=None,
                                in_=full[:, :],
                                in_offset=bass.IndirectOffsetOnAxis(
                                    ap=esrc_t[:, k : k + 1], axis=0
                                ),
                            )
                        # per-edge a_dst via transposed one-hots (exact f32)
                        ea_ps = pspool.tile([P, K_G * HEADS], F32, tag="ea", bufs=1)
                        for k in range(K_G):
                            row_ps = pspool.tile([P, P], F32, tag="tmp", bufs=2)
                            nc.tensor.transpose(
                                out=row_ps[:],
                                in_=dloc_t[:, k : k + 1].to_broadcast([P, P]),
                                identity=ident[:],
                            )
                            ohT = opool.tile([P, P], F32, tag="onehotT")
                            nc.vector.tensor_tensor(
                                out=ohT[:],
                                in0=iota_col[:].to_broadcast([P, P]),
                                in1=row_ps[:],
                                op=mybir.AluOpType.is_equal,
                            )
                            nc.tensor.matmul(
                                out=ea_ps[:, k * HEADS : (k + 1) * HEADS],
                                lhsT=ohT[:],
                                rhs=adst_all[:, t * HEADS : (t + 1) * HEADS],
                                start=True,
                                stop=True,
                            )
                        # alpha = (as_hi + as_lo) + ea   [P, K, 4] f32
                        a1 = spool.tile([P, K_G * HEADS], F32, tag="a1")
                        nc.vector.tensor_tensor(
                            out=a1[:].rearrange("p (k h) -> p k h", k=K_G),
                            in0=xa_v[:, :, HIDDEN : HIDDEN + HEADS],
                            in1=xa_v[:, :, HIDDEN + HEADS : HIDDEN + 2 * HEADS],
                            op=mybir.AluOpType.add,
                        )
                        alpha = spool.tile([P, K_G * HEADS], F32, tag="alpha")
                        nc.vector.tensor_add(alpha[:], a1[:], ea_ps[:])
                        # leaky relu: max(alpha, slope*alpha) on DVE
                        asc = spool.tile([P, K_G * HEADS], F32, tag="asc")
                        nc.vector.tensor_scalar_mul(asc[:], alpha[:], NEG_SLOPE)
                        lr = spool.tile([P, K_G * HEADS], F32, tag="lr")
                        nc.vector.tensor_tensor(
                            out=lr[:], in0=alpha[:], in1=asc[:], op=mybir.AluOpType.max
                        )
                        # merged rhs [msg(256) | ex(4)] per chunk
                        mg = mpool.tile([P, K_G * MW], BF16, tag="msg")
                        mg_v = mg[:].rearrange("p (k w) -> p k w", k=K_G)
                        nc.scalar.activation(
                            out=mg_v[:, :, HIDDEN:MW],
                            in_=lr[:].rearrange("p (k h) -> p k h", k=K_G),
                            func=mybir.ActivationFunctionType.Exp,
                        )
                        # ex expanded across the 64 head channels (ACT)
                        ex_rep = mpool.tile([P, K_G * HIDDEN], BF16, tag="ex_rep")
                        if probe.get("no_exprep"):
                            nc.vector.memset(ex_rep[:], 1.0)
                        else:
                            nc.scalar.activation(
                                out=ex_rep[:].rearrange(
                                    "p (k h c) -> p k h c", k=K_G, h=HEADS
                                ),
                                in_=lr[:]
                                .rearrange("p (k h) -> p k h", k=K_G)
                                .unsqueeze(-1)
                                .to_broadcast([P, K_G, HEADS, CH]),
                                func=mybir.ActivationFunctionType.Exp,
                            )
                        # msg = x * ex  (all-bf16 packed -> DVE 2x mode)
                        nc.vector.tensor_tensor(
                            out=mg_v[:, :, 0:HIDDEN],
                            in0=xa_v[:, :, 0:HIDDEN],
                            in1=ex_rep[:].rearrange("p (k c) -> p k c", k=K_G),
                            op=mybir.AluOpType.mult,
                        )
                        # one-hots
                        oh = opool.tile([P, K_G * P], BF16, tag="onehot")
                        nc.vector.tensor_tensor(
                            out=oh[:].rearrange("p (k e) -> p k e", k=K_G),
                            in0=dloc_bf[:].unsqueeze(-1).to_broadcast([P, K_G, P]),
                            in1=iota_row[:].unsqueeze(1).to_broadcast([P, K_G, P]),
                            op=mybir.AluOpType.is_equal,
                        )
                        # accumulation streak on PE: [num(256) | den(4)]
                        out_ps = pspool.tile([P, MW], F32, tag="out", bufs=2)
                        for k in range(K_G):
                            nc.tensor.matmul(
                                out=out_ps[:],
                                lhsT=oh[:, k * P : (k + 1) * P],
                                rhs=mg[:, k * MW : (k + 1) * MW],
                                start=(k == 0),
                                stop=(k == K_G - 1),
                            )
                        # normalize: h_next = num / den
                        den = spool.tile([P, HEADS], F32, tag="den")
                        nc.vector.tensor_scalar_add(
                            den[:], out_ps[:, HIDDEN:MW], 1e-30
                        )
                        dinv = spool.tile([P, HEADS], F32, tag="dinv")
                        nc.vector.reciprocal(dinv[:], den[:])
                        nc.vector.tensor_tensor(
                            out=hout[:, hcols(t)].rearrange("p (h c) -> p h c", h=HEADS),
                            in0=out_ps[:, 0:HIDDEN].rearrange("p (h c) -> p h c", h=HEADS),
                            in1=dinv[:].unsqueeze(-1).to_broadcast([P, HEADS, CH]),
                            op=mybir.AluOpType.mult,
                        )

                    if probe.get("stop_after") == f"gat{layer + 1}":
                        break
                if probe.get("stop_after") in ("dense1", "gat1", "dense2", "gat2"):
                    continue
                # ================= stage 5: score =================
                for t in range(T):
                    prod = spool.tile([P, HIDDEN], F32, tag="tsum")
                    nc.vector.tensor_mul(prod[:], hB[:, hcols(t)], worep_sb[:])
                    red = spool.tile([P, 1], F32, tag="red")
                    nc.vector.tensor_reduce(
                        out=red[:],
                        in_=prod[:],
                        axis=mybir.AxisListType.X,
                        op=mybir.AluOpType.add,
                    )
                    sc = spool.tile([P, 1], F32, tag="sc")
                    nc.vector.tensor_add(sc[:], red[:], scb_sb[:])
                    nc.sync.dma_start(score_out[t * P : (t + 1) * P], sc[:])

    nc.compile()
    return nc


# ---------------------------------------------------------------------------
# entry point
# ---------------------------------------------------------------------------

_CACHE = {}


def prepare(inputs, plan, probe=None):
    """Build (in_maps, nc, perm) from the full input dict + plan."""
    x = np.asarray(inputs["x"], np.float32)
    edge_type = np.asarray(inputs["edge_type"], np.int32)
    edge_weight = np.asarray(inputs["edge_weight"], np.float32)
    rel_emb = np.asarray(inputs["rel_emb"], np.float32)
    Wp = np.asarray(inputs["Wp"], np.float32)
    bp = np.asarray(inputs["bp"], np.float32)
    W1 = np.asarray(inputs["W1"], np.float32)
    W2 = np.asarray(inputs["W2"], np.float32)
    att_src1 = np.asarray(inputs["att_src1"], np.float32)
    att_dst1 = np.asarray(inputs["att_dst1"], np.float32)
    att_src2 = np.asarray(inputs["att_src2"], np.float32)
    att_dst2 = np.asarray(inputs["att_dst2"], np.float32)
    b1 = np.asarray(inputs["b1"], np.float32)
    b2 = np.asarray(inputs["b2"], np.float32)
    Wo = np.asarray(inputs["Wo"], np.float32)
    bo = np.asarray(inputs["bo"], np.float32)

    perm = plan["perm"]
    K_R, K_G = plan["K_R"], plan["K_G"]

    # ---- per-core dense inputs ----
    xr = np.concatenate([x[:, CODE_DIM:], CODE_WEIGHT * x[:, :CODE_DIM]], axis=1)
    xpad = np.zeros((NPAD, IN_DIM), np.float32)
    xpad[perm] = xr
    # [NCORES, T, KPROJ, P(feat), P(node)]
    xtt = (
        xpad.reshape(NCORES, T, P, KPROJ, P).transpose(0, 1, 3, 4, 2).copy()
    )

    w1aug = np.concatenate(
        [W1, W1 @ _asrc_mat(att_src1), W1 @ _asrc_mat(att_dst1)], axis=1
    )
    w2aug = np.concatenate(
        [W2, W2 @ _asrc_mat(att_src2), W2 @ _asrc_mat(att_dst2)], axis=1
    )
    b1w2 = (b1 @ w2aug).reshape(1, XAW).astype(np.float32)
    sc_bias = float(b2 @ Wo[:, 0] + bo[0])

    # ---- per-edge rel wtype rows: w_e * onehot6(type_e) ----
    eord_r = plan["eord_r"]  # [NTILES, P, K_R]
    wtyp = np.zeros((NTILES, P, K_R, NRELP), np.float32)
    valid = eord_r >= 0
    ew = np.where(valid, edge_weight[np.clip(eord_r, 0, E - 1)], 0.0).astype(np.float32)
    et = np.where(valid, edge_type[np.clip(eord_r, 0, E - 1)], 0)
    ii, jj, kk = np.nonzero(valid)
    wtyp[ii, jj, kk, et[ii, jj, kk]] = ew[ii, jj, kk]
    wtyp = wtyp.reshape(NTILES, P, K_R * NRELP)

    key = (K_R, K_G, tuple(sorted((probe or {}).items())))
    if key not in _CACHE:
        _CACHE[key] = _build_bass(K_R, K_G, probe)
    nc = _CACHE[key]

    common = dict(
        wp=np.ascontiguousarray(Wp.reshape(KPROJ, P, HIDDEN)),
        bp_row=bp.reshape(1, HIDDEN),
        w1aug=np.ascontiguousarray(w1aug.reshape(2, P, XAW)),
        w2aug=np.ascontiguousarray(w2aug.reshape(2, P, XAW)),
        b1w2_row=b1w2,
        rel_emb=np.concatenate(
            [rel_emb, np.zeros((NRELP - NREL, HIDDEN), np.float32)]
        ),
        wo_rep=np.ascontiguousarray(np.broadcast_to(Wo[:, 0], (P, HIDDEN))),
        sc_bias=np.full((P, 1), sc_bias, np.float32),
    )
    in_maps = []
    for c in range(NCORES):
        ts = slice(c * T, (c + 1) * T)
        in_maps.append(
            dict(
                common,
                xtt=xtt[c],
                esrc_r=plan["esrc_r"][ts],
                dloc_r=plan["dloc_r"][ts].astype(NPBF),
                wtyp_r=np.ascontiguousarray(wtyp[ts]).astype(NPBF),
                esrc_g=plan["esrc_g"][ts],
                dloc_g=plan["dloc_g"][ts],
            )
        )
    return in_maps, nc, perm


def kernel(x, edge_index, **rest):
    inputs = dict(rest, x=x, edge_index=edge_index)
    edge_index = np.asarray(edge_index, np.int32)
    plan = _build_plan(edge_index)
    in_maps, nc, perm = prepare(inputs, plan)

    import os

    trace = bool(os.environ.get("GAT_TRACE"))
    res = run_bass_kernel_spmd(
        nc, in_maps, core_ids=list(range(NCORES)), trace=trace
    )
    global _LAST_RESULT
    _LAST_RESULT = res
    scores_pad = np.concatenate([r["score"] for r in res.results])
    return scores_pad[perm].astype(np.float32)


_LAST_RESULT = None


# revision 75
# speedup vs baseline: 1.0730x; 1.0730x over previous
"""Trainium2 Bass kernel for nn_GATNodeScorer (GNN message passing).

Strategy (8 NeuronCores, node-partitioned):
  - Host: permute nodes into 160 balanced (core, tile) bins of 128 slots so
    every tile has <= K*128 in-edges; pack edges into 128-edge chunks per
    destination tile; fold attention projections and biases into augmented
    weight matrices.
  - Device, per core (SPMD, one NEFF):
      1. input projection  h = relu(xc @ Wp + bp)    (slab of 2560 nodes)
      2. AllGather H table (f32) across 8 cores
      3. relational layer  h1 = h + segsum(h[src] + rel_emb[type]*w)
         via per-chunk [P,1] indirect-DMA gathers + one-hot f32r matmul
         scatter-add
      4. dense x1 = h1 @ [W1 | W1@Asrc | W1@Adst] in f32r  -> XA table
         (bf16 x, attention logits as bf16 hi/lo pairs), AllGather
      5. GAT layer: per-chunk gathers of [x | as_hi | as_lo] rows by edge
         src; per-edge a_dst via transposed one-hot matmuls (exact f32);
         segment softmax via exp with the denominator columns merged into
         the numerator matmul rhs [msg(256) | ex(4)]; exp() is expanded
         across the 64 head channels on the ACT engine so the bf16 message
         scaling runs in DVE 2x mode
      6. repeat 4-5 for layer 2, then score = h3 @ Wo + bo
  - bf16 message data plane, f32r dense path, f32 PSUM accumulation.

Self-contained: hardcodes all shapes; only needs numpy + the concourse repo
installed at /opt/trn_rl_repo.
"""

import sys

sys.path.insert(0, "/opt/trn_rl_repo")

import heapq

import numpy as np
import ml_dtypes

import concourse.bass as bass
import concourse.bacc as bacc
import concourse.mybir as mybir
import concourse.tile as tile
from concourse.bass_utils import run_bass_kernel_spmd
from concourse.masks import make_identity

# ---- problem constants (hardcoded per contest rules) ----
N, E = 20000, 320000
IN_DIM, CODE_DIM, HIDDEN, HEADS, NREL = 896, 768, 256, 4, 5
CH = HIDDEN // HEADS
CODE_WEIGHT = 3.0
NEG_SLOPE = 0.2

NCORES = 8
P = 128
T = 20  # tiles per core
NTILES = NCORES * T  # 160
NP = T * P  # 2560 padded nodes per core
NPAD = NTILES * P  # 20480
KPROJ = IN_DIM // P  # 7

F32 = mybir.dt.float32
F32R = mybir.dt.float32r
BF16 = mybir.dt.bfloat16
I32 = mybir.dt.int32
NPBF = np.dtype(ml_dtypes.bfloat16)

XAW = HIDDEN + 2 * HEADS  # 264 dense output: [x | a_src | a_dst]
TBLW = HIDDEN + 2 * HEADS  # 264 shared table row: [x | as_hi | as_lo]
MW = HIDDEN + HEADS  # 260 merged matmul rhs: [msg | ex]
NRELP = 6  # NREL padded even

# ---------------------------------------------------------------------------
# host-side planning
# ---------------------------------------------------------------------------


def _pack_nodes(deg_gat, deg_rel, cap_g, cap_r):
    order = np.argsort(-deg_gat, kind="stable")
    load_g = np.zeros(NTILES, np.int64)
    load_r = np.zeros(NTILES, np.int64)
    count = np.zeros(NTILES, np.int64)
    tile_of = np.full(N, -1, np.int64)
    heap = [(0, t) for t in range(NTILES)]
    heapq.heapify(heap)
    for n in order:
        dg, dr = deg_gat[n], deg_rel[n]
        popped = []
        placed = False
        while heap:
            lg, t = heapq.heappop(heap)
            if lg != load_g[t]:
                continue
            if count[t] < P and load_g[t] + dg <= cap_g and load_r[t] + dr <= cap_r:
                tile_of[n] = t
                load_g[t] += dg
                load_r[t] += dr
                count[t] += 1
                if count[t] < P:
                    heapq.heappush(heap, (load_g[t], t))
                placed = True
                break
            popped.append((lg, t))
        for item in popped:
            heapq.heappush(heap, item)
        if not placed:
            raise RuntimeError("packing failed")
    return tile_of


def _pack_edges(src_pp, dst_pp, K):
    tile_e = dst_pp // P
    order_e = np.argsort(tile_e, kind="stable")
    esrc = np.zeros((NTILES, K * P), np.int32)
    dloc = np.full((NTILES, K * P), P, np.float32)
    eord = np.full((NTILES, K * P), -1, np.int64)
    bounds = np.searchsorted(tile_e[order_e], np.arange(NTILES + 1))
    for t in range(NTILES):
        lo, hi = bounds[t], bounds[t + 1]
        ecnt = hi - lo
        if ecnt > K * P:
            raise RuntimeError(f"tile {t}: {ecnt} edges > {K * P}")
        idxs = order_e[lo:hi]
        esrc[t, :ecnt] = src_pp[idxs]
        dloc[t, :ecnt] = (dst_pp[idxs] - t * P).astype(np.float32)
        eord[t, :ecnt] = idxs
    esrc = np.ascontiguousarray(esrc.reshape(NTILES, K, P).transpose(0, 2, 1))
    dloc = np.ascontiguousarray(dloc.reshape(NTILES, K, P).transpose(0, 2, 1))
    eord = np.ascontiguousarray(eord.reshape(NTILES, K, P).transpose(0, 2, 1))
    return esrc, dloc, eord


def _build_plan(edge_index):
    src = edge_index[0].astype(np.int64)
    dst = edge_index[1].astype(np.int64)
    deg_rel = np.bincount(dst, minlength=N)
    # GAT self-loops are handled analytically on-device (x, a_src, a_dst of
    # a node are all core-local), so the gathered edge set equals the rel
    # edge set and both stages share one table.
    for KK in (16, 17, 18):
        try:
            tile_of = _pack_nodes(deg_rel, deg_rel, KK * P, KK * P)
            break
        except RuntimeError:
            continue
    else:
        raise RuntimeError("node packing failed at all K")

    perm = np.full(N, -1, np.int64)
    slot_ctr = np.zeros(NTILES, np.int64)
    for n in np.argsort(tile_of, kind="stable"):
        t = tile_of[n]
        perm[n] = t * P + slot_ctr[t]
        slot_ctr[t] += 1

    src_p, dst_p = perm[src], perm[dst]
    esrc_r, dloc_r, eord_r = _pack_edges(src_p, dst_p, KK)
    return dict(
        perm=perm,
        K_G=KK,
        K_R=KK,
        esrc_r=esrc_r,
        dloc_r=dloc_r,
        eord_r=eord_r,
        esrc_g=esrc_r,
        dloc_g=dloc_r,
    )


def _asrc_mat(att):
    """[HEADS, CH] -> [HIDDEN, HEADS] block matrix so x @ A == (x*att).sum(-1)."""
    A = np.zeros((HIDDEN, HEADS), np.float32)
    for h in range(HEADS):
        A[h * CH : (h + 1) * CH, h] = att[h]
    return A


# ---------------------------------------------------------------------------
# bass program
# ---------------------------------------------------------------------------


def _build_bass(K_R, K_G, probe=None):
    probe = probe or {}
    reps = probe.get("reps", 1)
    nc = bacc.Bacc("TRN2", target_bir_lowering=False, debug=False, num_devices=NCORES)

    # ---- external inputs ----
    xtt_in = nc.dram_tensor("xtt", [T, KPROJ, P, P], F32, kind="ExternalInput")
    wp_in = nc.dram_tensor("wp", [KPROJ, P, HIDDEN], F32, kind="ExternalInput")
    bp_in = nc.dram_tensor("bp_row", [1, HIDDEN], F32, kind="ExternalInput")
    w1_in = nc.dram_tensor("w1aug", [2, P, XAW], F32, kind="ExternalInput")
    w2_in = nc.dram_tensor("w2aug", [2, P, XAW], F32, kind="ExternalInput")
    b1w2_in = nc.dram_tensor("b1w2_row", [1, XAW], F32, kind="ExternalInput")
    rel_in = nc.dram_tensor("rel_emb", [NRELP, HIDDEN], F32, kind="ExternalInput")
    worep_in = nc.dram_tensor("wo_rep", [P, HIDDEN], F32, kind="ExternalInput")
    scb_in = nc.dram_tensor("sc_bias", [P, 1], F32, kind="ExternalInput")
    esrc_r_in = nc.dram_tensor("esrc_r", [T, P, K_R], I32, kind="ExternalInput")
    dloc_r_in = nc.dram_tensor("dloc_r", [T, P, K_R], BF16, kind="ExternalInput")
    wtyp_r_in = nc.dram_tensor("wtyp_r", [T, P, NRELP * K_R], BF16, kind="ExternalInput")
    esrc_g_in = nc.dram_tensor("esrc_g", [T, P, K_G], I32, kind="ExternalInput")
    dloc_g_in = nc.dram_tensor("dloc_g", [T, P, K_G], F32, kind="ExternalInput")

    score_out = nc.dram_tensor("score", [NP], F32, kind="ExternalOutput")

    with tile.TileContext(nc) as tc:
        with (
            tc.tile_pool(name="const", bufs=1) as cpool,
            tc.tile_pool(name="hres", bufs=1) as hpool,
            tc.tile_pool(name="lhsT", bufs=4) as lpool,
            tc.tile_pool(name="edge_idx", bufs=3) as epool,
            tc.tile_pool(name="gather", bufs=3) as gpool,
            tc.tile_pool(name="onehot", bufs=2) as opool,
            tc.tile_pool(name="msg", bufs=2) as mpool,
            tc.tile_pool(name="small", bufs=4) as spool,
            tc.tile_pool(name="ps", bufs=1, space="PSUM") as pspool,
            tc.tile_pool(name="dram", bufs=1, space="DRAM") as dpool,
        ):
            # ---- constants ----
            ident = cpool.tile([P, P], F32)
            make_identity(nc, ident[:])
            ident_bf = cpool.tile([P, P], BF16)
            nc.vector.tensor_copy(ident_bf[:], ident[:])
            iota_row_i = cpool.tile([P, P], I32)
            nc.gpsimd.iota(iota_row_i[:], pattern=[[1, P]], base=0, channel_multiplier=0)
            iota_row = cpool.tile([P, P], BF16)
            nc.vector.tensor_copy(iota_row[:], iota_row_i[:])
            iota_row_f = cpool.tile([P, P], F32)
            nc.vector.tensor_copy(iota_row_f[:], iota_row_i[:])
            iota_col_i = cpool.tile([P, 1], I32)
            nc.gpsimd.iota(iota_col_i[:], pattern=[[0, 1]], base=0, channel_multiplier=1)
            iota_col = cpool.tile([P, 1], F32)
            nc.vector.tensor_copy(iota_col[:], iota_col_i[:])
            ones_row = cpool.tile([1, P], F32)
            nc.vector.memset(ones_row[:], 1.0)

            # weights resident in SBUF (f32r: full-rate matmuls, ~1e-5 error)
            w_scr = cpool.tile([P, 2 * XAW], F32)
            wp_sb = cpool.tile([P, KPROJ * HIDDEN], F32R)
            for k in range(KPROJ):
                nc.sync.dma_start(w_scr[:, 0:HIDDEN], wp_in[k, :, :])
                nc.vector.tensor_copy(
                    wp_sb[:, k * HIDDEN : (k + 1) * HIDDEN], w_scr[:, 0:HIDDEN]
                )
            bp_sb = cpool.tile([1, HIDDEN], F32)
            nc.sync.dma_start(bp_sb[:], bp_in[:, :])

            waug = []
            for li, w_in in enumerate((w1_in, w2_in)):
                wr = cpool.tile([P, 2 * XAW], F32R, name=f"w{li}")
                for k in range(2):
                    nc.sync.dma_start(w_scr[:, k * XAW : (k + 1) * XAW], w_in[k, :, :])
                nc.vector.tensor_copy(wr[:], w_scr[:])
                waug.append(wr)

            b1w2_sb = cpool.tile([1, XAW], F32)
            nc.sync.dma_start(b1w2_sb[:], b1w2_in[:, :])
            rel_f = cpool.tile([NRELP, HIDDEN], F32)
            nc.sync.dma_start(rel_f[:], rel_in[:, :])
            rel_sb = cpool.tile([NRELP, HIDDEN], BF16)
            nc.vector.tensor_copy(rel_sb[:], rel_f[:])
            worep_sb = cpool.tile([P, HIDDEN], F32)
            nc.sync.dma_start(worep_sb[:], worep_in[:, :])
            scb_sb = cpool.tile([P, 1], F32)
            nc.sync.dma_start(scb_sb[:], scb_in[:, :])

            # residual h slabs (two ping-pong slabs of T tiles, f32)
            hA = hpool.tile([P, T * HIDDEN], F32)
            hB = hpool.tile([P, T * HIDDEN], F32)
            # resident per-tile a_src/a_dst columns (f32, exact) and x rows
            # (bf16) for the analytic self-loop contribution
            adst_all = hpool.tile([P, T * HEADS], F32)
            asrc_all = hpool.tile([P, T * HEADS], F32)
            x_all = hpool.tile([P, T * HIDDEN], BF16)

            # DRAM bounce buffers for collectives (bf16).  A Shared tensor
            # may only be written by one instruction, so timing builds
            # (reps > 1) get per-rep tables.
            h_slab = dpool.tile([NP, HIDDEN], BF16)
            xa_slab = dpool.tile([NP, TBLW], BF16)
            xa_slab2 = dpool.tile([NP, TBLW], BF16)
            h_fulls = [
                dpool.tile([NPAD, HIDDEN], BF16, addr_space="Shared", name=f"h_full{r}")
                for r in range(reps)
            ]
            xa_fulls = [
                dpool.tile([NPAD, TBLW], BF16, addr_space="Shared", name=f"xa_full{r}")
                for r in range(reps)
            ]
            xa_full2s = [
                dpool.tile([NPAD, TBLW], BF16, addr_space="Shared", name=f"xa_full2{r}")
                for r in range(reps)
            ]

            def hcols(t):
                return slice(t * HIDDEN, (t + 1) * HIDDEN)

            for rep in range(reps):
                h_full = h_fulls[rep]
                xa_full = xa_fulls[rep]
                xa_full2 = xa_full2s[rep]
                # ================= stage 1: input projection =================
                for t in range(T):
                    proj_ps = pspool.tile([P, HIDDEN], F32, tag="work", bufs=2)
                    for k in range(KPROJ):
                        lx = lpool.tile([P, P], F32, tag="lhsT")
                        nc.sync.dma_start(lx[:], xtt_in[t, k, :, :])
                        lxr = lpool.tile([P, P], F32R, tag="lhsTr")
                        nc.vector.tensor_copy(lxr[:], lx[:])
                        nc.tensor.matmul(
                            out=proj_ps[:],
                            lhsT=lxr[:],
                            rhs=wp_sb[:, k * HIDDEN : (k + 1) * HIDDEN],
                            start=(k == 0),
                            stop=False,
                        )
                    nc.tensor.matmul(
                        out=proj_ps[:],
                        lhsT=ones_row[:1, :],
                        rhs=bp_sb[:1, :],
                        start=False,
                        stop=True,
                    )
                    nc.scalar.activation(
                        out=hA[:, hcols(t)],
                        in_=proj_ps[:],
                        func=mybir.ActivationFunctionType.Relu,
                    )
                    hsl = spool.tile([P, HIDDEN], BF16, tag="hsl")
                    nc.vector.tensor_copy(hsl[:], hA[:, hcols(t)])
                    nc.sync.dma_start(h_slab[t * P : (t + 1) * P, :], hsl[:])

                if probe.get("stop_after") == "proj":
                    continue
                # ================= AllGather H =================
                if probe.get("no_collective"):
                    nc.sync.dma_start(h_full[0:NP, :], h_slab[:, :])
                else:
                    nc.gpsimd.collective_compute(
                        "AllGather",
                        mybir.AluOpType.bypass,
                        replica_groups=[list(range(NCORES))],
                        ins=[h_slab.opt()],
                        outs=[h_full.opt()],
                    )

                # ================= stage 2: relational layer =================
                for t in range(T):
                    esrc_t = epool.tile([P, K_R], I32, tag="esrc")
                    nc.sync.dma_start(esrc_t[:], esrc_r_in[t, :, :])
                    dloc_t = epool.tile([P, K_R], BF16, tag="dlocb")
                    nc.sync.dma_start(dloc_t[:], dloc_r_in[t, :, :])
                    wt_t = epool.tile([P, NRELP * K_R], BF16, tag="wtyp")
                    nc.sync.dma_start(wt_t[:], wtyp_r_in[t, :, :])

                    # batched gather of all K_R chunks for this tile (bf16)
                    hch = gpool.tile([P, K_R * HIDDEN], BF16, tag="gather")
                    for k in range(K_R):
                        nc.gpsimd.indirect_dma_start(
                            out=hch[:, k * HIDDEN : (k + 1) * HIDDEN],
                            out_offset=None,
                            in_=h_full[:, :],
                            in_offset=bass.IndirectOffsetOnAxis(
                                ap=esrc_t[:, k : k + 1], axis=0
                            ),
                        )
                    # all one-hots in one DVE op
                    oh = opool.tile([P, K_R * P], BF16, tag="onehot")
                    nc.vector.tensor_tensor(
                        out=oh[:].rearrange("p (k e) -> p k e", k=K_R),
                        in0=dloc_t[:].unsqueeze(-1).to_broadcast([P, K_R, P]),
                        in1=iota_row[:].unsqueeze(1).to_broadcast([P, K_R, P]),
                        op=mybir.AluOpType.is_equal,
                    )
                    out_ps = pspool.tile([P, HIDDEN], F32, tag="out", bufs=2)
                    wm_ps = pspool.tile([P, NRELP], F32, tag="acc4", bufs=1)
                    for k in range(K_R):
                        nc.tensor.matmul(
                            out=out_ps[:],
                            lhsT=oh[:, k * P : (k + 1) * P],
                            rhs=hch[:, k * HIDDEN : (k + 1) * HIDDEN],
                            start=(k == 0),
                            stop=(k == K_R - 1),
                        )
                        nc.tensor.matmul(
                            out=wm_ps[:],
                            lhsT=oh[:, k * P : (k + 1) * P],
                            rhs=wt_t[:, k * NRELP : (k + 1) * NRELP],
                            start=(k == 0),
                            stop=(k == K_R - 1),
                        )
                    # rel contribution: wmatT [6, P] then rel_embT matmul
                    wmat_sb = spool.tile([P, NRELP], BF16, tag="wmat")
                    nc.vector.tensor_copy(wmat_sb[:], wm_ps[:])
                    wmatT_ps = pspool.tile([NRELP, P], BF16, tag="tmp", bufs=2)
                    nc.tensor.transpose(
                        out=wmatT_ps[:], in_=wmat_sb[:], identity=ident_bf[:]
                    )
                    wmatT_sb = spool.tile([NRELP, P], BF16, tag="wmatT")
                    nc.vector.tensor_copy(wmatT_sb[:], wmatT_ps[:])
                    rel_ps = pspool.tile([P, HIDDEN], F32, tag="work", bufs=2)
                    nc.tensor.matmul(
                        out=rel_ps[:],
                        lhsT=wmatT_sb[:],
                        rhs=rel_sb[:],
                        start=True,
                        stop=True,
                    )
                    # h1 = h + segsum + rel  (one PSUM operand per DVE op)
                    tsum = spool.tile([P, HIDDEN], F32, tag="tsum")
                    nc.vector.tensor_add(tsum[:], out_ps[:], hA[:, hcols(t)])
                    nc.vector.tensor_add(hB[:, hcols(t)], rel_ps[:], tsum[:])

                if probe.get("stop_after") == "rel":
                    continue
                # ============ stages 3/4: GAT layers ============
                for layer in range(2):
                    hin = hB if layer == 0 else hA
                    hout = hA if layer == 0 else hB
                    wr = waug[layer]
                    slab = xa_slab if layer == 0 else xa_slab2
                    full = xa_full if layer == 0 else xa_full2

                    # ---- dense: x = h @ Waug (+ b-fold for layer 1) ----
                    for t in range(T):
                        x_ps = pspool.tile([P, XAW], F32, tag="work", bufs=2)
                        for half in range(2):
                            tr_ps = pspool.tile([P, P], F32, tag="tmp", bufs=2)
                            nc.tensor.transpose(
                                out=tr_ps[:],
                                in_=hin[
                                    :,
                                    t * HIDDEN + half * P : t * HIDDEN + (half + 1) * P,
                                ],
                                identity=ident[:],
                            )
                            ht_r = lpool.tile([P, P], F32R, tag="lhsTr")
                            nc.vector.tensor_copy(ht_r[:], tr_ps[:])
                            nc.tensor.matmul(
                                out=x_ps[:],
                                lhsT=ht_r[:],
                                rhs=wr[:, half * XAW : (half + 1) * XAW],
                                start=(half == 0),
                                stop=(half == 1 and layer == 0),
                            )
                        if layer == 1:
                            # fold h2 = gat1_out + b1 into x2 = h2 @ W2aug
                            nc.tensor.matmul(
                                out=x_ps[:],
                                lhsT=ones_row[:1, :],
                                rhs=b1w2_sb[:1, :],
                                start=False,
                                stop=True,
                            )
                        # shared-table row [x(256) | as_hi | as_lo]; a_src is
                        # stored as a bf16 hi/lo split of the f32 logits
                        # (exp() amplifies rounding).  a_dst stays resident
                        # in f32 (only needed for local dst nodes).
                        xa_sb = gpool.tile([P, TBLW], BF16, tag="xa_sb")
                        nc.vector.tensor_copy(xa_sb[:, 0:HIDDEN], x_ps[:, 0:HIDDEN])
                        as_ps = x_ps[:, HIDDEN : HIDDEN + HEADS]
                        hi_ap = xa_sb[:, HIDDEN : HIDDEN + HEADS]
                        lo_ap = xa_sb[:, HIDDEN + HEADS : HIDDEN + 2 * HEADS]
                        nc.vector.tensor_copy(hi_ap, as_ps)
                        hi32 = spool.tile([P, HEADS], F32, tag="hi32")
                        nc.vector.tensor_copy(hi32[:], hi_ap)
                        nc.vector.tensor_tensor(
                            out=lo_ap, in0=as_ps, in1=hi32[:],
                            op=mybir.AluOpType.subtract,
                        )
                        nc.vector.tensor_copy(
                            adst_all[:, t * HEADS : (t + 1) * HEADS],
                            x_ps[:, HIDDEN + HEADS : XAW],
                        )
                        nc.vector.tensor_copy(
                            asrc_all[:, t * HEADS : (t + 1) * HEADS],
                            x_ps[:, HIDDEN : HIDDEN + HEADS],
                        )
                        nc.vector.tensor_copy(
                            x_all[:, hcols(t)], xa_sb[:, 0:HIDDEN]
                        )
                        nc.sync.dma_start(slab[t * P : (t + 1) * P, :], xa_sb[:])

                    if probe.get("no_collective"):
                        nc.sync.dma_start(full[0:NP, :], slab[:, :])
                    else:
                        nc.gpsimd.collective_compute(
                            "AllGather",
                            mybir.AluOpType.bypass,
                            replica_groups=[list(range(NCORES))],
                            ins=[slab.opt()],
                            outs=[full.opt()],
                        )

                    # ---- edge stage ----
                    if probe.get("stop_after") == f"dense{layer + 1}":
                        break
                    for t in range(T):
                        esrc_t = epool.tile([P, K_G], I32, tag="esrc")
                        nc.sync.dma_start(esrc_t[:], esrc_g_in[t, :, :])
                        dloc_t = epool.tile([P, K_G], F32, tag="dloc")
                        nc.sync.dma_start(dloc_t[:], dloc_g_in[t, :, :])
                        dloc_bf = epool.tile([P, K_G], BF16, tag="dlocb")
                        nc.vector.tensor_copy(dloc_bf[:], dloc_t[:])

                        # batched gather: [x | as_hi | as_lo] rows by src
                        xa_all = gpool.tile([P, K_G * TBLW], BF16, tag="gather")
                        xa_v = xa_all[:].rearrange("p (k w) -> p k w", k=K_G)
                        for k in range(K_G):
                            nc.gpsimd.indirect_dma_start(
                                out=xa_all[:, k * TBLW : (k + 1) * TBLW],
                                out_offset=None,
                                in_=full[:, :],
                                in_offset=bass.IndirectOffsetOnAxis(
                                    ap=esrc_t[:, k : k + 1], axis=0
                                ),
                            )
                        # per-edge a_dst via transposed one-hots (exact f32)
                        ea_ps = pspool.tile([P, K_G * HEADS], F32, tag="ea", bufs=1)
                        for k in range(K_G):
                            row_ps = pspool.tile([P, P], F32, tag="tmp", bufs=2)
                            nc.tensor.transpose(
                                out=row_ps[:],
                                in_=dloc_t[:, k : k + 1].to_broadcast([P, P]),
                                identity=ident[:],
                            )
                            ohT = opool.tile([P, P], F32, tag="onehotT")
                            nc.vector.tensor_tensor(
                                out=ohT[:],
                                in0=iota_col[:].to_broadcast([P, P]),
                                in1=row_ps[:],
                                op=mybir.AluOpType.is_equal,
                            )
                            nc.tensor.matmul(
                                out=ea_ps[:, k * HEADS : (k + 1) * HEADS],
                                lhsT=ohT[:],
                                rhs=adst_all[:, t * HEADS : (t + 1) * HEADS],
                                start=True,
                                stop=True,
                            )
                        # alpha = (as_hi + as_lo) + ea   [P, K, 4] f32
                        a1 = spool.tile([P, K_G * HEADS], F32, tag="a1")
                        nc.vector.tensor_tensor(
                            out=a1[:].rearrange("p (k h) -> p k h", k=K_G),
                            in0=xa_v[:, :, HIDDEN : HIDDEN + HEADS],
                            in1=xa_v[:, :, HIDDEN + HEADS : HIDDEN + 2 * HEADS],
                            op=mybir.AluOpType.add,
                        )
                        alpha = spool.tile([P, K_G * HEADS], F32, tag="alpha")
                        nc.vector.tensor_add(alpha[:], a1[:], ea_ps[:])
                        # leaky relu: max(alpha, slope*alpha) on DVE
                        asc = spool.tile([P, K_G * HEADS], F32, tag="asc")
                        nc.vector.tensor_scalar_mul(asc[:], alpha[:], NEG_SLOPE)
                        lr = spool.tile([P, K_G * HEADS], F32, tag="lr")
                        nc.vector.tensor_tensor(
                            out=lr[:], in0=alpha[:], in1=asc[:], op=mybir.AluOpType.max
                        )
                        # merged rhs [msg(256) | ex(4)] per chunk
                        mg = mpool.tile([P, K_G * MW], BF16, tag="msg")
                        mg_v = mg[:].rearrange("p (k w) -> p k w", k=K_G)
                        nc.scalar.activation(
                            out=mg_v[:, :, HIDDEN:MW],
                            in_=lr[:].rearrange("p (k h) -> p k h", k=K_G),
                            func=mybir.ActivationFunctionType.Exp,
                        )
                        # ex expanded across the 64 head channels (ACT)
                        ex_rep = mpool.tile([P, K_G * HIDDEN], BF16, tag="ex_rep")
                        if probe.get("no_exprep"):
                            nc.vector.memset(ex_rep[:], 1.0)
                        else:
                            nc.scalar.activation(
                                out=ex_rep[:].rearrange(
                                    "p (k h c) -> p k h c", k=K_G, h=HEADS
                                ),
                                in_=lr[:]
                                .rearrange("p (k h) -> p k h", k=K_G)
                                .unsqueeze(-1)
                                .to_broadcast([P, K_G, HEADS, CH]),
                                func=mybir.ActivationFunctionType.Exp,
                            )
                        # msg = x * ex  (all-bf16 packed -> DVE 2x mode)
                        nc.vector.tensor_tensor(
                            out=mg_v[:, :, 0:HIDDEN],
                            in0=xa_v[:, :, 0:HIDDEN],
                            in1=ex_rep[:].rearrange("p (k c) -> p k c", k=K_G),
                            op=mybir.AluOpType.mult,
                        )
                        # one-hots
                        oh = opool.tile([P, K_G * P], BF16, tag="onehot")
                        nc.vector.tensor_tensor(
                            out=oh[:].rearrange("p (k e) -> p k e", k=K_G),
                            in0=dloc_bf[:].unsqueeze(-1).to_broadcast([P, K_G, P]),
                            in1=iota_row[:].unsqueeze(1).to_broadcast([P, K_G, P]),
                            op=mybir.AluOpType.is_equal,
                        )
                        # accumulation streak on PE: [num(256) | den(4)]
                        out_ps = pspool.tile([P, MW], F32, tag="out", bufs=2)
                        for k in range(K_G):
                            nc.tensor.matmul(
                                out=out_ps[:],
                                lhsT=oh[:, k * P : (k + 1) * P],
                                rhs=mg[:, k * MW : (k + 1) * MW],
                                start=(k == 0),
                                stop=(k == K_G - 1),
                            )
                        # analytic self-loop contribution (x, a_src, a_dst all
                        # local; no gather, no one-hot, no edge slot)
                        a_s = spool.tile([P, HEADS], F32, tag="a_s")
                        nc.vector.tensor_add(
                            a_s[:],
                            asrc_all[:, t * HEADS : (t + 1) * HEADS],
                            adst_all[:, t * HEADS : (t + 1) * HEADS],
                        )
                        a_sc = spool.tile([P, HEADS], F32, tag="a_sc")
                        nc.vector.tensor_scalar_mul(a_sc[:], a_s[:], NEG_SLOPE)
                        lr_s = spool.tile([P, HEADS], F32, tag="lr_s")
                        nc.vector.tensor_tensor(
                            out=lr_s[:], in0=a_s[:], in1=a_sc[:],
                            op=mybir.AluOpType.max,
                        )
                        ex_s = spool.tile([P, HEADS], F32, tag="ex_s")
                        nc.scalar.activation(
                            out=ex_s[:],
                            in_=lr_s[:],
                            func=mybir.ActivationFunctionType.Exp,
                        )
                        smsg = spool.tile([P, MW], F32, tag="smsg")
                        nc.vector.tensor_tensor(
                            out=smsg[:, 0:HIDDEN].rearrange("p (h c) -> p h c", h=HEADS),
                            in0=x_all[:, hcols(t)].rearrange("p (h c) -> p h c", h=HEADS),
                            in1=ex_s[:].unsqueeze(-1).to_broadcast([P, HEADS, CH]),
                            op=mybir.AluOpType.mult,
                        )
                        nc.vector.tensor_copy(smsg[:, HIDDEN:MW], ex_s[:])
                        tot = spool.tile([P, MW], F32, tag="tot")
                        nc.vector.tensor_add(tot[:], out_ps[:], smsg[:])
                        # normalize: h_next = num / den
                        den = spool.tile([P, HEADS], F32, tag="den")
                        nc.vector.tensor_scalar_add(
                            den[:], tot[:, HIDDEN:MW], 1e-30
                        )
                        dinv = spool.tile([P, HEADS], F32, tag="dinv")
                        nc.vector.reciprocal(dinv[:], den[:])
                        nc.vector.tensor_tensor(
                            out=hout[:, hcols(t)].rearrange("p (h c) -> p h c", h=HEADS),
                            in0=tot[:, 0:HIDDEN].rearrange("p (h c) -> p h c", h=HEADS),
                            in1=dinv[:].unsqueeze(-1).to_broadcast([P, HEADS, CH]),
                            op=mybir.AluOpType.mult,
                        )

                    if probe.get("stop_after") == f"gat{layer + 1}":
                        break
                if probe.get("stop_after") in ("dense1", "gat1", "dense2", "gat2"):
                    continue
                # ================= stage 5: score =================
                for t in range(T):
                    prod = spool.tile([P, HIDDEN], F32, tag="tsum")
                    nc.vector.tensor_mul(prod[:], hB[:, hcols(t)], worep_sb[:])
                    red = spool.tile([P, 1], F32, tag="red")
                    nc.vector.tensor_reduce(
                        out=red[:],
                        in_=prod[:],
                        axis=mybir.AxisListType.X,
                        op=mybir.AluOpType.add,
                    )
                    sc = spool.tile([P, 1], F32, tag="sc")
                    nc.vector.tensor_add(sc[:], red[:], scb_sb[:])
                    nc.sync.dma_start(score_out[t * P : (t + 1) * P], sc[:])

    nc.compile()
    return nc


# ---------------------------------------------------------------------------
# entry point
# ---------------------------------------------------------------------------

_CACHE = {}


def prepare(inputs, plan, probe=None):
    """Build (in_maps, nc, perm) from the full input dict + plan."""
    x = np.asarray(inputs["x"], np.float32)
    edge_type = np.asarray(inputs["edge_type"], np.int32)
    edge_weight = np.asarray(inputs["edge_weight"], np.float32)
    rel_emb = np.asarray(inputs["rel_emb"], np.float32)
    Wp = np.asarray(inputs["Wp"], np.float32)
    bp = np.asarray(inputs["bp"], np.float32)
    W1 = np.asarray(inputs["W1"], np.float32)
    W2 = np.asarray(inputs["W2"], np.float32)
    att_src1 = np.asarray(inputs["att_src1"], np.float32)
    att_dst1 = np.asarray(inputs["att_dst1"], np.float32)
    att_src2 = np.asarray(inputs["att_src2"], np.float32)
    att_dst2 = np.asarray(inputs["att_dst2"], np.float32)
    b1 = np.asarray(inputs["b1"], np.float32)
    b2 = np.asarray(inputs["b2"], np.float32)
    Wo = np.asarray(inputs["Wo"], np.float32)
    bo = np.asarray(inputs["bo"], np.float32)

    perm = plan["perm"]
    K_R, K_G = plan["K_R"], plan["K_G"]

    # ---- per-core dense inputs ----
    xr = np.concatenate([x[:, CODE_DIM:], CODE_WEIGHT * x[:, :CODE_DIM]], axis=1)
    xpad = np.zeros((NPAD, IN_DIM), np.float32)
    xpad[perm] = xr
    # [NCORES, T, KPROJ, P(feat), P(node)]
    xtt = (
        xpad.reshape(NCORES, T, P, KPROJ, P).transpose(0, 1, 3, 4, 2).copy()
    )

    w1aug = np.concatenate(
        [W1, W1 @ _asrc_mat(att_src1), W1 @ _asrc_mat(att_dst1)], axis=1
    )
    w2aug = np.concatenate(
        [W2, W2 @ _asrc_mat(att_src2), W2 @ _asrc_mat(att_dst2)], axis=1
    )
    b1w2 = (b1 @ w2aug).reshape(1, XAW).astype(np.float32)
    sc_bias = float(b2 @ Wo[:, 0] + bo[0])

    # ---- per-edge rel wtype rows: w_e * onehot6(type_e) ----
    eord_r = plan["eord_r"]  # [NTILES, P, K_R]
    wtyp = np.zeros((NTILES, P, K_R, NRELP), np.float32)
    valid = eord_r >= 0
    ew = np.where(valid, edge_weight[np.clip(eord_r, 0, E - 1)], 0.0).astype(np.float32)
    et = np.where(valid, edge_type[np.clip(eord_r, 0, E - 1)], 0)
    ii, jj, kk = np.nonzero(valid)
    wtyp[ii, jj, kk, et[ii, jj, kk]] = ew[ii, jj, kk]
    wtyp = wtyp.reshape(NTILES, P, K_R * NRELP)

    key = (K_R, K_G, tuple(sorted((probe or {}).items())))
    if key not in _CACHE:
        _CACHE[key] = _build_bass(K_R, K_G, probe)
    nc = _CACHE[key]

    common = dict(
        wp=np.ascontiguousarray(Wp.reshape(KPROJ, P, HIDDEN)),
        bp_row=bp.reshape(1, HIDDEN),
        w1aug=np.ascontiguousarray(w1aug.reshape(2, P, XAW)),
        w2aug=np.ascontiguousarray(w2aug.reshape(2, P, XAW)),
        b1w2_row=b1w2,
        rel_emb=np.concatenate(
            [rel_emb, np.zeros((NRELP - NREL, HIDDEN), np.float32)]
        ),
        wo_rep=np.ascontiguousarray(np.broadcast_to(Wo[:, 0], (P, HIDDEN))),
        sc_bias=np.full((P, 1), sc_bias, np.float32),
    )
    in_maps = []
    for c in range(NCORES):
        ts = slice(c * T, (c + 1) * T)
        in_maps.append(
            dict(
                common,
                xtt=xtt[c],
                esrc_r=plan["esrc_r"][ts],
                dloc_r=plan["dloc_r"][ts].astype(NPBF),
                wtyp_r=np.ascontiguousarray(wtyp[ts]).astype(NPBF),
                esrc_g=plan["esrc_g"][ts],
                dloc_g=plan["dloc_g"][ts],
            )
        )
    return in_maps, nc, perm


def kernel(x, edge_index, **rest):
    inputs = dict(rest, x=x, edge_index=edge_index)
    edge_index = np.asarray(edge_index, np.int32)
    plan = _build_plan(edge_index)
    in_maps, nc, perm = prepare(inputs, plan)

    import os

    trace = bool(os.environ.get("GAT_TRACE"))
    res = run_bass_kernel_spmd(
        nc, in_maps, core_ids=list(range(NCORES)), trace=trace
    )
    global _LAST_RESULT
    _LAST_RESULT = res
    scores_pad = np.concatenate([r["score"] for r in res.results])
    return scores_pad[perm].astype(np.float32)


_LAST_RESULT = None


# revision 80
# speedup vs baseline: 1.1159x; 1.0401x over previous
"""Trainium2 Bass kernel for nn_GATNodeScorer (GNN message passing).

Strategy (8 NeuronCores, node-partitioned):
  - Host: permute nodes into 160 balanced (core, tile) bins of 128 slots so
    every tile has <= K*128 in-edges; pack edges into 128-edge chunks per
    destination tile; fold attention projections and biases into augmented
    weight matrices.
  - Device, per core (SPMD, one NEFF):
      1. input projection  h = relu(xc @ Wp + bp)    (slab of 2560 nodes)
      2. AllGather H table (f32) across 8 cores
      3. relational layer  h1 = h + segsum(h[src] + rel_emb[type]*w)
         via per-chunk [P,1] indirect-DMA gathers + one-hot f32r matmul
         scatter-add
      4. dense x1 = h1 @ [W1 | W1@Asrc | W1@Adst] in f32r  -> XA table
         (bf16 x, attention logits as bf16 hi/lo pairs), AllGather
      5. GAT layer: per-chunk gathers of [x | as_hi | as_lo] rows by edge
         src; per-edge a_dst via transposed one-hot matmuls (exact f32);
         segment softmax via exp with the denominator columns merged into
         the numerator matmul rhs [msg(256) | ex(4)]; exp() is expanded
         across the 64 head channels on the ACT engine so the bf16 message
         scaling runs in DVE 2x mode
      6. repeat 4-5 for layer 2, then score = h3 @ Wo + bo
  - bf16 message data plane, f32r dense path, f32 PSUM accumulation.

Self-contained: hardcodes all shapes; only needs numpy + the concourse repo
installed at /opt/trn_rl_repo.
"""

import sys

sys.path.insert(0, "/opt/trn_rl_repo")

import heapq

import numpy as np
import ml_dtypes

import concourse.bass as bass
import concourse.bacc as bacc
import concourse.mybir as mybir
import concourse.tile as tile
from concourse.bass_utils import run_bass_kernel_spmd
from concourse.masks import make_identity

# ---- problem constants (hardcoded per contest rules) ----
N, E = 20000, 320000
IN_DIM, CODE_DIM, HIDDEN, HEADS, NREL = 896, 768, 256, 4, 5
CH = HIDDEN // HEADS
CODE_WEIGHT = 3.0
NEG_SLOPE = 0.2

NCORES = 8
P = 128
T = 20  # tiles per core
NTILES = NCORES * T  # 160
NP = T * P  # 2560 padded nodes per core
NPAD = NTILES * P  # 20480
KPROJ = IN_DIM // P  # 7

F32 = mybir.dt.float32
F32R = mybir.dt.float32r
BF16 = mybir.dt.bfloat16
I32 = mybir.dt.int32
NPBF = np.dtype(ml_dtypes.bfloat16)

XAW = HIDDEN + 2 * HEADS  # 264 dense output: [x | a_src | a_dst]
TBLW = HIDDEN + 2 * HEADS  # 264 shared table row: [x | as_hi | as_lo]
MW = HIDDEN + HEADS  # 260 merged matmul rhs: [msg | ex]
NRELP = 6  # NREL padded even

# ---------------------------------------------------------------------------
# host-side planning
# ---------------------------------------------------------------------------


def _pack_nodes(deg_gat, deg_rel, cap_g, cap_r):
    order = np.argsort(-deg_gat, kind="stable")
    load_g = np.zeros(NTILES, np.int64)
    load_r = np.zeros(NTILES, np.int64)
    count = np.zeros(NTILES, np.int64)
    tile_of = np.full(N, -1, np.int64)
    heap = [(0, t) for t in range(NTILES)]
    heapq.heapify(heap)
    for n in order:
        dg, dr = deg_gat[n], deg_rel[n]
        popped = []
        placed = False
        while heap:
            lg, t = heapq.heappop(heap)
            if lg != load_g[t]:
                continue
            if count[t] < P and load_g[t] + dg <= cap_g and load_r[t] + dr <= cap_r:
                tile_of[n] = t
                load_g[t] += dg
                load_r[t] += dr
                count[t] += 1
                if count[t] < P:
                    heapq.heappush(heap, (load_g[t], t))
                placed = True
                break
            popped.append((lg, t))
        for item in popped:
            heapq.heappush(heap, item)
        if not placed:
            raise RuntimeError("packing failed")
    return tile_of


def _pack_edges(src_pp, dst_pp, K):
    tile_e = dst_pp // P
    order_e = np.argsort(tile_e, kind="stable")
    esrc = np.zeros((NTILES, K * P), np.int32)
    dloc = np.full((NTILES, K * P), P, np.float32)
    eord = np.full((NTILES, K * P), -1, np.int64)
    bounds = np.searchsorted(tile_e[order_e], np.arange(NTILES + 1))
    for t in range(NTILES):
        lo, hi = bounds[t], bounds[t + 1]
        ecnt = hi - lo
        if ecnt > K * P:
            raise RuntimeError(f"tile {t}: {ecnt} edges > {K * P}")
        idxs = order_e[lo:hi]
        esrc[t, :ecnt] = src_pp[idxs]
        dloc[t, :ecnt] = (dst_pp[idxs] - t * P).astype(np.float32)
        eord[t, :ecnt] = idxs
    esrc = np.ascontiguousarray(esrc.reshape(NTILES, K, P).transpose(0, 2, 1))
    dloc = np.ascontiguousarray(dloc.reshape(NTILES, K, P).transpose(0, 2, 1))
    eord = np.ascontiguousarray(eord.reshape(NTILES, K, P).transpose(0, 2, 1))
    return esrc, dloc, eord


def _build_plan(edge_index):
    src = edge_index[0].astype(np.int64)
    dst = edge_index[1].astype(np.int64)
    deg_rel = np.bincount(dst, minlength=N)
    # GAT self-loops are handled analytically on-device (x, a_src, a_dst of
    # a node are all core-local), so the gathered edge set equals the rel
    # edge set and both stages share one table.
    for KK in (16, 17, 18):
        try:
            tile_of = _pack_nodes(deg_rel, deg_rel, KK * P, KK * P)
            break
        except RuntimeError:
            continue
    else:
        raise RuntimeError("node packing failed at all K")

    perm = np.full(N, -1, np.int64)
    slot_ctr = np.zeros(NTILES, np.int64)
    for n in np.argsort(tile_of, kind="stable"):
        t = tile_of[n]
        perm[n] = t * P + slot_ctr[t]
        slot_ctr[t] += 1

    src_p, dst_p = perm[src], perm[dst]
    esrc_r, dloc_r, eord_r = _pack_edges(src_p, dst_p, KK)
    return dict(
        perm=perm,
        K_G=KK,
        K_R=KK,
        esrc_r=esrc_r,
        dloc_r=dloc_r,
        eord_r=eord_r,
        esrc_g=esrc_r,
        dloc_g=dloc_r,
    )


def _asrc_mat(att):
    """[HEADS, CH] -> [HIDDEN, HEADS] block matrix so x @ A == (x*att).sum(-1)."""
    A = np.zeros((HIDDEN, HEADS), np.float32)
    for h in range(HEADS):
        A[h * CH : (h + 1) * CH, h] = att[h]
    return A


# ---------------------------------------------------------------------------
# bass program
# ---------------------------------------------------------------------------


def _build_bass(K_R, K_G, probe=None):
    probe = probe or {}
    reps = probe.get("reps", 1)
    nc = bacc.Bacc("TRN2", target_bir_lowering=False, debug=False, num_devices=NCORES)

    # ---- external inputs ----
    xtt_in = nc.dram_tensor("xtt", [T, KPROJ, P, P], BF16, kind="ExternalInput")
    wp_in = nc.dram_tensor("wp", [KPROJ, P, HIDDEN], BF16, kind="ExternalInput")
    bp_in = nc.dram_tensor("bp_row", [1, HIDDEN], F32, kind="ExternalInput")
    w1_in = nc.dram_tensor("w1aug", [2, P, XAW], F32, kind="ExternalInput")
    w2_in = nc.dram_tensor("w2aug", [2, P, XAW], F32, kind="ExternalInput")
    b1w2_in = nc.dram_tensor("b1w2_row", [1, XAW], F32, kind="ExternalInput")
    rel_in = nc.dram_tensor("rel_emb", [NRELP, HIDDEN], F32, kind="ExternalInput")
    worep_in = nc.dram_tensor("wo_rep", [P, HIDDEN], F32, kind="ExternalInput")
    scb_in = nc.dram_tensor("sc_bias", [P, 1], F32, kind="ExternalInput")
    esrc_r_in = nc.dram_tensor("esrc_r", [T, P, K_R], I32, kind="ExternalInput")
    dloc_r_in = nc.dram_tensor("dloc_r", [T, P, K_R], BF16, kind="ExternalInput")
    wtyp_r_in = nc.dram_tensor("wtyp_r", [T, P, NRELP * K_R], BF16, kind="ExternalInput")
    esrc_g_in = nc.dram_tensor("esrc_g", [T, P, K_G], I32, kind="ExternalInput")
    dloc_g_in = nc.dram_tensor("dloc_g", [T, P, K_G], F32, kind="ExternalInput")

    score_out = nc.dram_tensor("score", [NP], F32, kind="ExternalOutput")

    with tile.TileContext(nc) as tc:
        with (
            tc.tile_pool(name="const", bufs=1) as cpool,
            tc.tile_pool(name="hres", bufs=1) as hpool,
            tc.tile_pool(name="lhsT", bufs=4) as lpool,
            tc.tile_pool(name="edge_idx", bufs=3) as epool,
            tc.tile_pool(name="gather", bufs=3) as gpool,
            tc.tile_pool(name="onehot", bufs=2) as opool,
            tc.tile_pool(name="msg", bufs=2) as mpool,
            tc.tile_pool(name="small", bufs=4) as spool,
            tc.tile_pool(name="ps", bufs=1, space="PSUM") as pspool,
            tc.tile_pool(name="dram", bufs=1, space="DRAM") as dpool,
        ):
            # ---- constants ----
            ident = cpool.tile([P, P], F32)
            make_identity(nc, ident[:])
            ident_bf = cpool.tile([P, P], BF16)
            nc.vector.tensor_copy(ident_bf[:], ident[:])
            iota_row_i = cpool.tile([P, P], I32)
            nc.gpsimd.iota(iota_row_i[:], pattern=[[1, P]], base=0, channel_multiplier=0)
            iota_row = cpool.tile([P, P], BF16)
            nc.vector.tensor_copy(iota_row[:], iota_row_i[:])
            iota_row_f = cpool.tile([P, P], F32)
            nc.vector.tensor_copy(iota_row_f[:], iota_row_i[:])
            iota_col_i = cpool.tile([P, 1], I32)
            nc.gpsimd.iota(iota_col_i[:], pattern=[[0, 1]], base=0, channel_multiplier=1)
            iota_col = cpool.tile([P, 1], F32)
            nc.vector.tensor_copy(iota_col[:], iota_col_i[:])
            ones_row = cpool.tile([1, P], F32)
            nc.vector.memset(ones_row[:], 1.0)

            # weights resident in SBUF (proj bf16; GAT dense f32r since exp
            # amplifies its rounding)
            w_scr = cpool.tile([P, 2 * XAW], F32)
            wp_sb = cpool.tile([P, KPROJ * HIDDEN], BF16)
            for k in range(KPROJ):
                nc.sync.dma_start(
                    wp_sb[:, k * HIDDEN : (k + 1) * HIDDEN], wp_in[k, :, :]
                )
            bp_sb = cpool.tile([1, HIDDEN], F32)
            nc.sync.dma_start(bp_sb[:], bp_in[:, :])

            waug = []
            for li, w_in in enumerate((w1_in, w2_in)):
                wr = cpool.tile([P, 2 * XAW], F32R, name=f"w{li}")
                for k in range(2):
                    nc.sync.dma_start(w_scr[:, k * XAW : (k + 1) * XAW], w_in[k, :, :])
                nc.vector.tensor_copy(wr[:], w_scr[:])
                waug.append(wr)

            b1w2_sb = cpool.tile([1, XAW], F32)
            nc.sync.dma_start(b1w2_sb[:], b1w2_in[:, :])
            rel_f = cpool.tile([NRELP, HIDDEN], F32)
            nc.sync.dma_start(rel_f[:], rel_in[:, :])
            rel_sb = cpool.tile([NRELP, HIDDEN], BF16)
            nc.vector.tensor_copy(rel_sb[:], rel_f[:])
            worep_sb = cpool.tile([P, HIDDEN], F32)
            nc.sync.dma_start(worep_sb[:], worep_in[:, :])
            scb_sb = cpool.tile([P, 1], F32)
            nc.sync.dma_start(scb_sb[:], scb_in[:, :])

            # residual h slabs (two ping-pong slabs of T tiles, f32)
            hA = hpool.tile([P, T * HIDDEN], F32)
            hB = hpool.tile([P, T * HIDDEN], F32)
            # resident per-tile a_src/a_dst columns (f32, exact) and x rows
            # (bf16) for the analytic self-loop contribution
            adst_all = hpool.tile([P, T * HEADS], F32)
            asrc_all = hpool.tile([P, T * HEADS], F32)
            x_all = hpool.tile([P, T * HIDDEN], BF16)

            # DRAM bounce buffers for collectives (bf16).  A Shared tensor
            # may only be written by one instruction, so timing builds
            # (reps > 1) get per-rep tables.
            h_slab = dpool.tile([NP, HIDDEN], BF16)
            xa_slab = dpool.tile([NP, TBLW], BF16)
            xa_slab2 = dpool.tile([NP, TBLW], BF16)
            h_fulls = [
                dpool.tile([NPAD, HIDDEN], BF16, addr_space="Shared", name=f"h_full{r}")
                for r in range(reps)
            ]
            xa_fulls = [
                dpool.tile([NPAD, TBLW], BF16, addr_space="Shared", name=f"xa_full{r}")
                for r in range(reps)
            ]
            xa_full2s = [
                dpool.tile([NPAD, TBLW], BF16, addr_space="Shared", name=f"xa_full2{r}")
                for r in range(reps)
            ]

            def hcols(t):
                return slice(t * HIDDEN, (t + 1) * HIDDEN)

            for rep in range(reps):
                h_full = h_fulls[rep]
                xa_full = xa_fulls[rep]
                xa_full2 = xa_full2s[rep]
                # ================= stage 1: input projection =================
                for t in range(T):
                    proj_ps = pspool.tile([P, HIDDEN], F32, tag="work", bufs=2)
                    for k in range(KPROJ):
                        lx = lpool.tile([P, P], BF16, tag="lhsT")
                        nc.sync.dma_start(lx[:], xtt_in[t, k, :, :])
                        nc.tensor.matmul(
                            out=proj_ps[:],
                            lhsT=lx[:],
                            rhs=wp_sb[:, k * HIDDEN : (k + 1) * HIDDEN],
                            start=(k == 0),
                            stop=False,
                        )
                    nc.tensor.matmul(
                        out=proj_ps[:],
                        lhsT=ones_row[:1, :],
                        rhs=bp_sb[:1, :],
                        start=False,
                        stop=True,
                    )
                    nc.scalar.activation(
                        out=hA[:, hcols(t)],
                        in_=proj_ps[:],
                        func=mybir.ActivationFunctionType.Relu,
                    )
                    hsl = spool.tile([P, HIDDEN], BF16, tag="hsl")
                    nc.vector.tensor_copy(hsl[:], hA[:, hcols(t)])
                    nc.sync.dma_start(h_slab[t * P : (t + 1) * P, :], hsl[:])

                if probe.get("stop_after") == "proj":
                    continue
                # ================= AllGather H =================
                if probe.get("no_collective"):
                    nc.sync.dma_start(h_full[0:NP, :], h_slab[:, :])
                else:
                    nc.gpsimd.collective_compute(
                        "AllGather",
                        mybir.AluOpType.bypass,
                        replica_groups=[list(range(NCORES))],
                        ins=[h_slab.opt()],
                        outs=[h_full.opt()],
                    )

                # ================= stage 2: relational layer =================
                for t in range(T):
                    esrc_t = epool.tile([P, K_R], I32, tag="esrc")
                    nc.sync.dma_start(esrc_t[:], esrc_r_in[t, :, :])
                    dloc_t = epool.tile([P, K_R], BF16, tag="dlocb")
                    nc.sync.dma_start(dloc_t[:], dloc_r_in[t, :, :])
                    wt_t = epool.tile([P, NRELP * K_R], BF16, tag="wtyp")
                    nc.sync.dma_start(wt_t[:], wtyp_r_in[t, :, :])

                    # batched gather of all K_R chunks for this tile (bf16)
                    hch = gpool.tile([P, K_R * HIDDEN], BF16, tag="gather")
                    for k in range(K_R):
                        nc.gpsimd.indirect_dma_start(
                            out=hch[:, k * HIDDEN : (k + 1) * HIDDEN],
                            out_offset=None,
                            in_=h_full[:, :],
                            in_offset=bass.IndirectOffsetOnAxis(
                                ap=esrc_t[:, k : k + 1], axis=0
                            ),
                        )
                    # all one-hots in one DVE op
                    oh = opool.tile([P, K_R * P], BF16, tag="onehot")
                    nc.vector.tensor_tensor(
                        out=oh[:].rearrange("p (k e) -> p k e", k=K_R),
                        in0=dloc_t[:].unsqueeze(-1).to_broadcast([P, K_R, P]),
                        in1=iota_row[:].unsqueeze(1).to_broadcast([P, K_R, P]),
                        op=mybir.AluOpType.is_equal,
                    )
                    out_ps = pspool.tile([P, HIDDEN], F32, tag="out", bufs=2)
                    wm_ps = pspool.tile([P, NRELP], F32, tag="acc4", bufs=1)
                    for k in range(K_R):
                        nc.tensor.matmul(
                            out=out_ps[:],
                            lhsT=oh[:, k * P : (k + 1) * P],
                            rhs=hch[:, k * HIDDEN : (k + 1) * HIDDEN],
                            start=(k == 0),
                            stop=(k == K_R - 1),
                        )
                        nc.tensor.matmul(
                            out=wm_ps[:],
                            lhsT=oh[:, k * P : (k + 1) * P],
                            rhs=wt_t[:, k * NRELP : (k + 1) * NRELP],
                            start=(k == 0),
                            stop=(k == K_R - 1),
                        )
                    # rel contribution: wmatT [6, P] then rel_embT matmul
                    wmat_sb = spool.tile([P, NRELP], BF16, tag="wmat")
                    nc.vector.tensor_copy(wmat_sb[:], wm_ps[:])
                    wmatT_ps = pspool.tile([NRELP, P], BF16, tag="tmp", bufs=2)
                    nc.tensor.transpose(
                        out=wmatT_ps[:], in_=wmat_sb[:], identity=ident_bf[:]
                    )
                    wmatT_sb = spool.tile([NRELP, P], BF16, tag="wmatT")
                    nc.vector.tensor_copy(wmatT_sb[:], wmatT_ps[:])
                    rel_ps = pspool.tile([P, HIDDEN], F32, tag="work", bufs=2)
                    nc.tensor.matmul(
                        out=rel_ps[:],
                        lhsT=wmatT_sb[:],
                        rhs=rel_sb[:],
                        start=True,
                        stop=True,
                    )
                    # h1 = h + segsum + rel  (one PSUM operand per DVE op)
                    tsum = spool.tile([P, HIDDEN], F32, tag="tsum")
                    nc.vector.tensor_add(tsum[:], out_ps[:], hA[:, hcols(t)])
                    nc.vector.tensor_add(hB[:, hcols(t)], rel_ps[:], tsum[:])

                if probe.get("stop_after") == "rel":
                    continue
                # ============ stages 3/4: GAT layers ============
                for layer in range(2):
                    hin = hB if layer == 0 else hA
                    hout = hA if layer == 0 else hB
                    wr = waug[layer]
                    slab = xa_slab if layer == 0 else xa_slab2
                    full = xa_full if layer == 0 else xa_full2

                    # ---- dense: x = h @ Waug (+ b-fold for layer 1) ----
                    for t in range(T):
                        x_ps = pspool.tile([P, XAW], F32, tag="work", bufs=2)
                        for half in range(2):
                            tr_ps = pspool.tile([P, P], F32, tag="tmp", bufs=2)
                            nc.tensor.transpose(
                                out=tr_ps[:],
                                in_=hin[
                                    :,
                                    t * HIDDEN + half * P : t * HIDDEN + (half + 1) * P,
                                ],
                                identity=ident[:],
                            )
                            ht_r = lpool.tile([P, P], F32R, tag="lhsTr")
                            nc.vector.tensor_copy(ht_r[:], tr_ps[:])
                            nc.tensor.matmul(
                                out=x_ps[:],
                                lhsT=ht_r[:],
                                rhs=wr[:, half * XAW : (half + 1) * XAW],
                                start=(half == 0),
                                stop=(half == 1 and layer == 0),
                            )
                        if layer == 1:
                            # fold h2 = gat1_out + b1 into x2 = h2 @ W2aug
                            nc.tensor.matmul(
                                out=x_ps[:],
                                lhsT=ones_row[:1, :],
                                rhs=b1w2_sb[:1, :],
                                start=False,
                                stop=True,
                            )
                        # shared-table row [x(256) | as_hi | as_lo]; a_src is
                        # stored as a bf16 hi/lo split of the f32 logits
                        # (exp() amplifies rounding).  a_dst stays resident
                        # in f32 (only needed for local dst nodes).
                        xa_sb = gpool.tile([P, TBLW], BF16, tag="xa_sb")
                        nc.vector.tensor_copy(xa_sb[:, 0:HIDDEN], x_ps[:, 0:HIDDEN])
                        as_ps = x_ps[:, HIDDEN : HIDDEN + HEADS]
                        hi_ap = xa_sb[:, HIDDEN : HIDDEN + HEADS]
                        lo_ap = xa_sb[:, HIDDEN + HEADS : HIDDEN + 2 * HEADS]
                        nc.vector.tensor_copy(hi_ap, as_ps)
                        hi32 = spool.tile([P, HEADS], F32, tag="hi32")
                        nc.vector.tensor_copy(hi32[:], hi_ap)
                        nc.vector.tensor_tensor(
                            out=lo_ap, in0=as_ps, in1=hi32[:],
                            op=mybir.AluOpType.subtract,
                        )
                        nc.vector.tensor_copy(
                            adst_all[:, t * HEADS : (t + 1) * HEADS],
                            x_ps[:, HIDDEN + HEADS : XAW],
                        )
                        nc.vector.tensor_copy(
                            asrc_all[:, t * HEADS : (t + 1) * HEADS],
                            x_ps[:, HIDDEN : HIDDEN + HEADS],
                        )
                        nc.vector.tensor_copy(
                            x_all[:, hcols(t)], xa_sb[:, 0:HIDDEN]
                        )
                        nc.sync.dma_start(slab[t * P : (t + 1) * P, :], xa_sb[:])

                    if probe.get("no_collective"):
                        nc.sync.dma_start(full[0:NP, :], slab[:, :])
                    else:
                        nc.gpsimd.collective_compute(
                            "AllGather",
                            mybir.AluOpType.bypass,
                            replica_groups=[list(range(NCORES))],
                            ins=[slab.opt()],
                            outs=[full.opt()],
                        )

                    # ---- edge stage ----
                    if probe.get("stop_after") == f"dense{layer + 1}":
                        break
                    for t in range(T):
                        esrc_t = epool.tile([P, K_G], I32, tag="esrc")
                        nc.sync.dma_start(esrc_t[:], esrc_g_in[t, :, :])
                        dloc_t = epool.tile([P, K_G], F32, tag="dloc")
                        nc.sync.dma_start(dloc_t[:], dloc_g_in[t, :, :])
                        dloc_bf = epool.tile([P, K_G], BF16, tag="dlocb")
                        nc.vector.tensor_copy(dloc_bf[:], dloc_t[:])

                        # batched gather: [x | as_hi | as_lo] rows by src
                        xa_all = gpool.tile([P, K_G * TBLW], BF16, tag="gather")
                        xa_v = xa_all[:].rearrange("p (k w) -> p k w", k=K_G)
                        for k in range(K_G):
                            nc.gpsimd.indirect_dma_start(
                                out=xa_all[:, k * TBLW : (k + 1) * TBLW],
                                out_offset=None,
                                in_=full[:, :],
                                in_offset=bass.IndirectOffsetOnAxis(
                                    ap=esrc_t[:, k : k + 1], axis=0
                                ),
                            )
                        # per-edge a_dst via transposed one-hots (exact f32)
                        ea_ps = pspool.tile([P, K_G * HEADS], F32, tag="ea", bufs=1)
                        for k in range(K_G):
                            row_ps = pspool.tile([P, P], F32, tag="tmp", bufs=2)
                            nc.tensor.transpose(
                                out=row_ps[:],
                                in_=dloc_t[:, k : k + 1].to_broadcast([P, P]),
                                identity=ident[:],
                            )
                            ohT = opool.tile([P, P], F32, tag="onehotT")
                            nc.vector.tensor_tensor(
                                out=ohT[:],
                                in0=iota_col[:].to_broadcast([P, P]),
                                in1=row_ps[:],
                                op=mybir.AluOpType.is_equal,
                            )
                            nc.tensor.matmul(
                                out=ea_ps[:, k * HEADS : (k + 1) * HEADS],
                                lhsT=ohT[:],
                                rhs=adst_all[:, t * HEADS : (t + 1) * HEADS],
                                start=True,
                                stop=True,
                            )
                        # alpha = (as_hi + as_lo) + ea   [P, K, 4] f32
                        a1 = spool.tile([P, K_G * HEADS], F32, tag="a1")
                        nc.vector.tensor_tensor(
                            out=a1[:].rearrange("p (k h) -> p k h", k=K_G),
                            in0=xa_v[:, :, HIDDEN : HIDDEN + HEADS],
                            in1=xa_v[:, :, HIDDEN + HEADS : HIDDEN + 2 * HEADS],
                            op=mybir.AluOpType.add,
                        )
                        alpha = spool.tile([P, K_G * HEADS], F32, tag="alpha")
                        nc.vector.tensor_add(alpha[:], a1[:], ea_ps[:])
                        # leaky relu: max(alpha, slope*alpha) on DVE
                        asc = spool.tile([P, K_G * HEADS], F32, tag="asc")
                        nc.vector.tensor_scalar_mul(asc[:], alpha[:], NEG_SLOPE)
                        lr = spool.tile([P, K_G * HEADS], F32, tag="lr")
                        nc.vector.tensor_tensor(
                            out=lr[:], in0=alpha[:], in1=asc[:], op=mybir.AluOpType.max
                        )
                        # merged rhs [msg(256) | ex(4)] per chunk
                        mg = mpool.tile([P, K_G * MW], BF16, tag="msg")
                        mg_v = mg[:].rearrange("p (k w) -> p k w", k=K_G)
                        nc.scalar.activation(
                            out=mg_v[:, :, HIDDEN:MW],
                            in_=lr[:].rearrange("p (k h) -> p k h", k=K_G),
                            func=mybir.ActivationFunctionType.Exp,
                        )
                        # ex expanded across the 64 head channels (ACT)
                        ex_rep = mpool.tile([P, K_G * HIDDEN], BF16, tag="ex_rep")
                        if probe.get("no_exprep"):
                            nc.vector.memset(ex_rep[:], 1.0)
                        else:
                            nc.scalar.activation(
                                out=ex_rep[:].rearrange(
                                    "p (k h c) -> p k h c", k=K_G, h=HEADS
                                ),
                                in_=lr[:]
                                .rearrange("p (k h) -> p k h", k=K_G)
                                .unsqueeze(-1)
                                .to_broadcast([P, K_G, HEADS, CH]),
                                func=mybir.ActivationFunctionType.Exp,
                            )
                        # msg = x * ex  (all-bf16 packed -> DVE 2x mode)
                        nc.vector.tensor_tensor(
                            out=mg_v[:, :, 0:HIDDEN],
                            in0=xa_v[:, :, 0:HIDDEN],
                            in1=ex_rep[:].rearrange("p (k c) -> p k c", k=K_G),
                            op=mybir.AluOpType.mult,
                        )
                        # one-hots
                        oh = opool.tile([P, K_G * P], BF16, tag="onehot")
                        nc.vector.tensor_tensor(
                            out=oh[:].rearrange("p (k e) -> p k e", k=K_G),
                            in0=dloc_bf[:].unsqueeze(-1).to_broadcast([P, K_G, P]),
                            in1=iota_row[:].unsqueeze(1).to_broadcast([P, K_G, P]),
                            op=mybir.AluOpType.is_equal,
                        )
                        # accumulation streak on PE: [num(256) | den(4)]
                        out_ps = pspool.tile([P, MW], F32, tag="out", bufs=2)
                        for k in range(K_G):
                            nc.tensor.matmul(
                                out=out_ps[:],
                                lhsT=oh[:, k * P : (k + 1) * P],
                                rhs=mg[:, k * MW : (k + 1) * MW],
                                start=(k == 0),
                                stop=(k == K_G - 1),
                            )
                        # analytic self-loop contribution (x, a_src, a_dst all
                        # local; no gather, no one-hot, no edge slot)
                        a_s = spool.tile([P, HEADS], F32, tag="a_s")
                        nc.vector.tensor_add(
                            a_s[:],
                            asrc_all[:, t * HEADS : (t + 1) * HEADS],
                            adst_all[:, t * HEADS : (t + 1) * HEADS],
                        )
                        a_sc = spool.tile([P, HEADS], F32, tag="a_sc")
                        nc.vector.tensor_scalar_mul(a_sc[:], a_s[:], NEG_SLOPE)
                        lr_s = spool.tile([P, HEADS], F32, tag="lr_s")
                        nc.vector.tensor_tensor(
                            out=lr_s[:], in0=a_s[:], in1=a_sc[:],
                            op=mybir.AluOpType.max,
                        )
                        ex_s = spool.tile([P, HEADS], F32, tag="ex_s")
                        nc.scalar.activation(
                            out=ex_s[:],
                            in_=lr_s[:],
                            func=mybir.ActivationFunctionType.Exp,
                        )
                        smsg = spool.tile([P, MW], F32, tag="smsg")
                        nc.vector.tensor_tensor(
                            out=smsg[:, 0:HIDDEN].rearrange("p (h c) -> p h c", h=HEADS),
                            in0=x_all[:, hcols(t)].rearrange("p (h c) -> p h c", h=HEADS),
                            in1=ex_s[:].unsqueeze(-1).to_broadcast([P, HEADS, CH]),
                            op=mybir.AluOpType.mult,
                        )
                        nc.vector.tensor_copy(smsg[:, HIDDEN:MW], ex_s[:])
                        tot = spool.tile([P, MW], F32, tag="tot")
                        nc.vector.tensor_add(tot[:], out_ps[:], smsg[:])
                        # normalize: h_next = num / den
                        den = spool.tile([P, HEADS], F32, tag="den")
                        nc.vector.tensor_scalar_add(
                            den[:], tot[:, HIDDEN:MW], 1e-30
                        )
                        dinv = spool.tile([P, HEADS], F32, tag="dinv")
                        nc.vector.reciprocal(dinv[:], den[:])
                        nc.vector.tensor_tensor(
                            out=hout[:, hcols(t)].rearrange("p (h c) -> p h c", h=HEADS),
                            in0=tot[:, 0:HIDDEN].rearrange("p (h c) -> p h c", h=HEADS),
                            in1=dinv[:].unsqueeze(-1).to_broadcast([P, HEADS, CH]),
                            op=mybir.AluOpType.mult,
                        )

                    if probe.get("stop_after") == f"gat{layer + 1}":
                        break
                if probe.get("stop_after") in ("dense1", "gat1", "dense2", "gat2"):
                    continue
                # ================= stage 5: score =================
                for t in range(T):
                    prod = spool.tile([P, HIDDEN], F32, tag="tsum")
                    nc.vector.tensor_mul(prod[:], hB[:, hcols(t)], worep_sb[:])
                    red = spool.tile([P, 1], F32, tag="red")
                    nc.vector.tensor_reduce(
                        out=red[:],
                        in_=prod[:],
                        axis=mybir.AxisListType.X,
                        op=mybir.AluOpType.add,
                    )
                    sc = spool.tile([P, 1], F32, tag="sc")
                    nc.vector.tensor_add(sc[:], red[:], scb_sb[:])
                    nc.sync.dma_start(score_out[t * P : (t + 1) * P], sc[:])

    nc.compile()
    return nc


# ---------------------------------------------------------------------------
# entry point
# ---------------------------------------------------------------------------

_CACHE = {}


def prepare(inputs, plan, probe=None):
    """Build (in_maps, nc, perm) from the full input dict + plan."""
    x = np.asarray(inputs["x"], np.float32)
    edge_type = np.asarray(inputs["edge_type"], np.int32)
    edge_weight = np.asarray(inputs["edge_weight"], np.float32)
    rel_emb = np.asarray(inputs["rel_emb"], np.float32)
    Wp = np.asarray(inputs["Wp"], np.float32)
    bp = np.asarray(inputs["bp"], np.float32)
    W1 = np.asarray(inputs["W1"], np.float32)
    W2 = np.asarray(inputs["W2"], np.float32)
    att_src1 = np.asarray(inputs["att_src1"], np.float32)
    att_dst1 = np.asarray(inputs["att_dst1"], np.float32)
    att_src2 = np.asarray(inputs["att_src2"], np.float32)
    att_dst2 = np.asarray(inputs["att_dst2"], np.float32)
    b1 = np.asarray(inputs["b1"], np.float32)
    b2 = np.asarray(inputs["b2"], np.float32)
    Wo = np.asarray(inputs["Wo"], np.float32)
    bo = np.asarray(inputs["bo"], np.float32)

    perm = plan["perm"]
    K_R, K_G = plan["K_R"], plan["K_G"]

    # ---- per-core dense inputs ----
    xr = np.concatenate([x[:, CODE_DIM:], CODE_WEIGHT * x[:, :CODE_DIM]], axis=1)
    xpad = np.zeros((NPAD, IN_DIM), np.float32)
    xpad[perm] = xr
    # [NCORES, T, KPROJ, P(feat), P(node)]
    xtt = (
        xpad.reshape(NCORES, T, P, KPROJ, P).transpose(0, 1, 3, 4, 2).astype(NPBF)
    )

    w1aug = np.concatenate(
        [W1, W1 @ _asrc_mat(att_src1), W1 @ _asrc_mat(att_dst1)], axis=1
    )
    w2aug = np.concatenate(
        [W2, W2 @ _asrc_mat(att_src2), W2 @ _asrc_mat(att_dst2)], axis=1
    )
    b1w2 = (b1 @ w2aug).reshape(1, XAW).astype(np.float32)
    sc_bias = float(b2 @ Wo[:, 0] + bo[0])

    # ---- per-edge rel wtype rows: w_e * onehot6(type_e) ----
    eord_r = plan["eord_r"]  # [NTILES, P, K_R]
    wtyp = np.zeros((NTILES, P, K_R, NRELP), np.float32)
    valid = eord_r >= 0
    ew = np.where(valid, edge_weight[np.clip(eord_r, 0, E - 1)], 0.0).astype(np.float32)
    et = np.where(valid, edge_type[np.clip(eord_r, 0, E - 1)], 0)
    ii, jj, kk = np.nonzero(valid)
    wtyp[ii, jj, kk, et[ii, jj, kk]] = ew[ii, jj, kk]
    wtyp = wtyp.reshape(NTILES, P, K_R * NRELP)

    key = (K_R, K_G, tuple(sorted((probe or {}).items())))
    if key not in _CACHE:
        _CACHE[key] = _build_bass(K_R, K_G, probe)
    nc = _CACHE[key]

    common = dict(
        wp=np.ascontiguousarray(Wp.reshape(KPROJ, P, HIDDEN)).astype(NPBF),
        bp_row=bp.reshape(1, HIDDEN),
        w1aug=np.ascontiguousarray(w1aug.reshape(2, P, XAW)),
        w2aug=np.ascontiguousarray(w2aug.reshape(2, P, XAW)),
        b1w2_row=b1w2,
        rel_emb=np.concatenate(
            [rel_emb, np.zeros((NRELP - NREL, HIDDEN), np.float32)]
        ),
        wo_rep=np.ascontiguousarray(np.broadcast_to(Wo[:, 0], (P, HIDDEN))),
        sc_bias=np.full((P, 1), sc_bias, np.float32),
    )
    in_maps = []
    for c in range(NCORES):
        ts = slice(c * T, (c + 1) * T)
        in_maps.append(
            dict(
                common,
                xtt=xtt[c],
                esrc_r=plan["esrc_r"][ts],
                dloc_r=plan["dloc_r"][ts].astype(NPBF),
                wtyp_r=np.ascontiguousarray(wtyp[ts]).astype(NPBF),
                esrc_g=plan["esrc_g"][ts],
                dloc_g=plan["dloc_g"][ts],
            )
        )
    return in_maps, nc, perm


def kernel(x, edge_index, **rest):
    inputs = dict(rest, x=x, edge_index=edge_index)
    edge_index = np.asarray(edge_index, np.int32)
    plan = _build_plan(edge_index)
    in_maps, nc, perm = prepare(inputs, plan)

    import os

    trace = bool(os.environ.get("GAT_TRACE"))
    res = run_bass_kernel_spmd(
        nc, in_maps, core_ids=list(range(NCORES)), trace=trace
    )
    global _LAST_RESULT
    _LAST_RESULT = res
    scores_pad = np.concatenate([r["score"] for r in res.results])
    return scores_pad[perm].astype(np.float32)


_LAST_RESULT = None
